# revision 1
# baseline (speedup 1.0000x reference)
"""Trainium2 Bass kernel for nn_Decoder (MLP -> inverse token embedding ->
overlap-add -> channel-merge conv), data-parallel over batch on 8 NeuronCores.

Self-contained: hardcodes shapes; host-side numpy folds everything after the
first Linear+ReLU into per-channel fused matrices G (W2 -> Winv -> overlap-add
normalization -> 3-tap channel conv), so the device pipeline is:

    x[tok,E] --PE transpose--> xT[E,tok] --matmul W1T--> h[Hc,tok] in PSUM
    --ACT/DVE relu+bias--> hT in SBUF --matmul G (accum over c,Hc)--> v[66,tok]
    --PE transpose--> vT[b,66] --strided DVE adds (overlap-add)--> y[b,1056]

Sharding: batch 1024 -> 8 cores x 128.
"""

import numpy as np

import concourse.bacc as bacc
import concourse.mybir as mybir
from concourse.bass_utils import run_bass_kernel_spmd
from concourse.tile import TileContext

# problem shapes (hardcoded per contract)
B, C, T, E, H = 1024, 8, 32, 128, 256
SEG_LEN, SIG_LEN, NUM_SEG, STEP = 64, 1056, 32, 32
N_CORES = 8
BL = B // N_CORES          # local batch per core = 128
HC = H // 128              # H chunks = 2
TC = 8                     # t-chunks
TL = T // TC               # t per chunk = 4
FD = mybir.dt.float32
FR = mybir.dt.float32r   # fp32 storage, FP22 multiply: 4x faster PE
FH = mybir.dt.float16
X16 = True               # load x as fp16 via DMA-transpose (no PE transposes)

_CACHE = {}


def _host_prep(W1, b1, W2, b2, Winv, binv, Wconv, bconv):
    """Fold W2/Winv/normalization/conv into G [3var][C][H,66] and bias B[1056]."""
    counter = np.zeros(SIG_LEN, np.float64)
    for t in range(NUM_SEG):
        counter[t * STEP: t * STEP + SEG_LEN] += 1.0
    n = 1.0 / counter

    F = Winv.astype(np.float64) @ W2.astype(np.float64)          # [64, H]
    binv2 = Winv.astype(np.float64) @ b2.astype(np.float64) + binv.astype(np.float64)
    Wc = Wconv[0].astype(np.float64)                             # [C, 3]

    def n_of(var, s):
        if var == 0:
            return n[s]
        if var == 2:
            return n[992 + s]
        return 0.5

    G = np.zeros((3, C, H, 66), np.float64)
    for var in range(3):
        for c in range(C):
            for m_idx in range(66):
                for k in range(3):
                    s = m_idx + k - 2
                    if 0 <= s < SEG_LEN:
                        G[var, c, :, m_idx] += Wc[c, k] * n_of(var, s) * F[s, :]

    sig_b = np.zeros(SIG_LEN, np.float64)
    for t in range(NUM_SEG):
        sig_b[t * STEP: t * STEP + SEG_LEN] += binv2
    sig_b *= n
    Bvec = np.full(SIG_LEN, float(np.asarray(bconv).reshape(-1)[0]), np.float64)
    q = np.arange(SIG_LEN)
    for k in range(3):
        qq = q + k - 1
        valid = (qq >= 0) & (qq < SIG_LEN)
        for c in range(C):
            Bvec[valid] += Wc[c, k] * sig_b[qq[valid]]
    return G.astype(np.float32), Bvec.astype(np.float32)


def _g_col(hc, c, var):
    """Column offset of G slice (hc, c, var) inside g_sb [128, 2*8*3*66]."""
    return ((hc * C + c) * 3 + var) * 66


def _build_bass(debug=False, x16=X16):
    nc = bacc.Bacc("TRN2")

    if x16:
        # host pre-transposed to [C, T, BL, E] fp16 so each (c, t-chunk) is a
        # contiguous 2D block for the xbar DMA-transpose
        x = nc.dram_tensor("x", [C, T, BL, E], FH, kind="ExternalInput")
    else:
        x = nc.dram_tensor("x", [BL, C, T, E], FR, kind="ExternalInput")
    w1t = nc.dram_tensor("w1t", [E, H], FH if x16 else FR, kind="ExternalInput")
    b1c = nc.dram_tensor("b1c", [128, HC], FD, kind="ExternalInput")
    g = nc.dram_tensor("g", [128, HC * C * 3 * 66], FH if x16 else FR,
                       kind="ExternalInput")
    brep = nc.dram_tensor("brep", [BL, SIG_LEN], FD, kind="ExternalInput")
    ident = nc.dram_tensor("ident", [128, 128], FR, kind="ExternalInput")
    y = nc.dram_tensor("y", [BL, SIG_LEN], FD, kind="ExternalOutput")
    if debug:
        dbg_xt = nc.dram_tensor("dbg_xt", [128, TL * 128], FH if x16 else FR,
                                kind="ExternalOutput")
        dbg_ht = nc.dram_tensor("dbg_ht", [HC, 128, C * TL * 128], FH if x16 else FR,
                                kind="ExternalOutput")
        dbg_v = nc.dram_tensor("dbg_v", [BL, T * 66], FD, kind="ExternalOutput")

    with TileContext(nc) as tc:
        with (
            tc.tile_pool(name="consts", bufs=1) as consts,
            tc.tile_pool(name="xin", bufs=6) as xin_pool,
            tc.tile_pool(name="xt", bufs=18) as xt_pool,
            tc.tile_pool(name="ht", bufs=2) as ht_pool,
            tc.tile_pool(name="vsb", bufs=3) as vsb_pool,
            tc.tile_pool(name="big", bufs=1) as big_pool,
            tc.tile_pool(name="pe_out", bufs=1, space="PSUM") as peout_pool,
            tc.tile_pool(name="h_ps", bufs=4, space="PSUM") as hps_pool,
            tc.tile_pool(name="v_ps", bufs=3, space="PSUM") as vps_pool,
        ):
            w1t_sb = consts.tile([E, H], FH if x16 else FR)
            b1c_sb = consts.tile([128, HC], FD)

            def emit_w1_loads():
                nc.sync.dma_start(out=w1t_sb[:], in_=w1t[:])
                nc.sync.dma_start(out=b1c_sb[:], in_=b1c[:])
            g_sb = consts.tile([128, HC * C * 3 * 66], FH if x16 else FR)
            ident_sb = consts.tile([128, 128], FR)
            brep_sb = big_pool.tile([BL, SIG_LEN], FD)

            def emit_const_loads():
                # emitted after the first x DMATs so they don't hog HWDGE
                nc.sync.dma_start(out=g_sb[:], in_=g[:])
                nc.sync.dma_start(out=ident_sb[:], in_=ident[:])
                nc.sync.dma_start(out=brep_sb[:], in_=brep[:])

            V_sb = big_pool.tile([BL, T * 66], FD)      # v transposed: [b, t*66+m]
            y_sb = big_pool.tile([BL, SIG_LEN], FD)

            # software pipeline: fused stage runs one t-chunk behind MLP1
            ht_tiles = {}

            # greedy ACT/DVE load balancer for PSUM->SBUF copies and relus
            eng_busy = {"act": 0.0, "dve": 0.0}

            def pick_engine(fd):
                ca = (172 + fd) / 1.2
                cd = (120 + fd) / 0.96
                if eng_busy["act"] + ca <= eng_busy["dve"] + cd:
                    eng_busy["act"] += ca
                    return "act"
                eng_busy["dve"] += cd
                return "dve"

            def bal_copy(out, in_, fd):
                if pick_engine(fd) == "act":
                    nc.scalar.copy(out=out, in_=in_)
                else:
                    nc.vector.tensor_copy(out=out, in_=in_)

            def chunk_ranges(tcix):
                # column ranges with uniform G variant; cols = tl*128 + b
                if tcix == 0:
                    return [(0, 128, 0), (128, 512, 1)]       # t=0 -> var 0
                if tcix == TC - 1:
                    return [(0, 384, 1), (384, 512, 2)]       # t=31 -> var 2
                return [(0, 512, 1)]

            def emit_loads_transposes(tcix):
                xt_list = []
                for c in range(C):
                    if x16:
                        # xbar DMA-transpose: [(tl,b), e] -> [e, tl*128+b]
                        xt_sb = xt_pool.tile([128, TL * 128], FH, tag="xt")
                        src_rows = x[c, tcix * TL:(tcix + 1) * TL, :, :]
                        nc.sync.dma_start_transpose(
                            out=xt_sb[:],
                            in_=src_rows.rearrange("t b e -> (t b) e"),
                        )
                        xt_list.append(xt_sb)
                        if debug and tcix == 0 and c == 0:
                            nc.sync.dma_start(out=dbg_xt[:], in_=xt_sb[:])
                        continue
                    # load x block: [b=128 part, (tl, e)]
                    xtile = xin_pool.tile([BL, TL, E], FR, tag="xin")
                    nc.sync.dma_start(
                        out=xtile[:],
                        in_=x[:, c, tcix * TL:(tcix + 1) * TL, :],
                    )
                    # PE transpose each [b, e] slice -> xT [e, tl*128 + b]
                    xt_ps = peout_pool.tile([128, TL * 128], FR, tag="pe_out")
                    for tl in range(TL):
                        nc.tensor.transpose(
                            xt_ps[:, tl * 128:(tl + 1) * 128],
                            xtile[:, tl, :],
                            ident_sb[:],
                        )
                    xt_sb = xt_pool.tile([128, TL * 128], FR, tag="xt")
                    nc.scalar.copy(out=xt_sb[:], in_=xt_ps[:])
                    xt_list.append(xt_sb)
                    if debug and tcix == 0 and c == 0:
                        nc.sync.dma_start(out=dbg_xt[:], in_=xt_sb[:])
                return xt_list

            def emit_mlp1(tcix, xt_list, c):
                ht = ht_tiles[tcix]
                xt_sb = xt_list[c]
                h_list = []
                for hc in range(HC):
                    h_ps = hps_pool.tile([128, TL * 128], FD, tag="h_ps",
                                         name=f"h_ps_{tcix}_{c}_{hc}")
                    nc.tensor.matmul(
                        h_ps[:],
                        w1t_sb[:, hc * 128:(hc + 1) * 128],
                        xt_sb[:],
                        start=True, stop=True,
                    )
                    h_list.append(h_ps)
                for hc in range(HC):
                    # relu + bias -> hT slice; alternate ACT/DVE engines
                    dst = ht[(c, hc)][:]
                    src = h_list[hc][:]
                    if pick_engine(TL * 128) == "act":
                        nc.scalar.activation(
                            dst, src,
                            mybir.ActivationFunctionType.Relu,
                            bias=b1c_sb[:, hc:hc + 1], scale=1.0,
                        )
                    else:
                        nc.vector.tensor_scalar(
                            dst, src,
                            b1c_sb[:, hc:hc + 1], 0.0,
                            mybir.AluOpType.add, mybir.AluOpType.max,
                        )

            def emit_fused(tcix, v_tiles, c):
                """fused G matmuls for channel c accumulating into v_tiles."""
                ht = ht_tiles[tcix]
                for (lo, hi, var, v_ps) in v_tiles:
                    for hc in range(HC):
                        i = c * HC + hc
                        nc.tensor.matmul(
                            v_ps[:, lo:hi],
                            g_sb[:, _g_col(hc, c, var):_g_col(hc, c, var) + 66],
                            ht[(c, hc)][:, lo:hi],
                            start=(i == 0), stop=(i == C * HC - 1),
                        )

            def emit_vtrans(tcix, v_tiles):
                """copy v psum -> sbuf, PE-transpose per t into V_sb."""
                if debug and tcix == 0:
                    for hc in range(HC):
                        for c in range(C):
                            nc.sync.dma_start(
                                out=dbg_ht[hc][:, c * 512:(c + 1) * 512],
                                in_=ht_tiles[0][(c, hc)][:])
                del ht_tiles[tcix]
                v_sb = vsb_pool.tile([66, 512], FR, tag="v_sb")
                for (lo, hi, var, v_ps) in v_tiles:
                    bal_copy(v_sb[:, lo:hi], v_ps[:, lo:hi], hi - lo)
                for tl in range(TL):
                    t = tcix * TL + tl
                    vt_ps = peout_pool.tile([128, 66], FR, tag="pe_out")
                    nc.tensor.transpose(
                        vt_ps[:],
                        v_sb[:, tl * 128:(tl + 1) * 128],
                        ident_sb[0:66, 0:66],
                    )
                    bal_copy(V_sb[:, t * 66:(t + 1) * 66], vt_ps[:], 66)

            # overlap-add assembly in rounds (per watermark) so it overlaps
            # with later chunks instead of serializing at the end
            V3 = V_sb[:].rearrange("b (t m) -> b t m", m=66)
            Y3 = y_sb[:].rearrange("b (j r) -> b j r", r=32)
            B3 = brep_sb[:].rearrange("b (j r) -> b j r", r=32)

            def emit_y_assembly(j_lo, j_hi):
                """Assemble y blocks j in [j_lo, j_hi); requires V[t] for
                t <= j_hi (uses t=j+1 for the r=31 edge). Runs on GpSimd
                (SBUF-only) to keep DVE/ACT free for PSUM drains."""
                eng = nc.gpsimd
                jm = min(j_hi, 32)      # main1 defined for j<=31
                if jm > j_lo:
                    eng.tensor_add(
                        out=Y3[:, j_lo:jm, :], in0=V3[:, j_lo:jm, 1:33],
                        in1=B3[:, j_lo:jm, :])
                if j_hi == 33:          # last block: bias only here
                    eng.tensor_copy(
                        out=y_sb[:, 1024:1056], in_=brep_sb[:, 1024:1056])
                lo = max(1, j_lo)
                if j_hi > lo:           # += v[:, j-1, r+33]
                    eng.tensor_add(
                        out=Y3[:, lo:j_hi, :], in0=Y3[:, lo:j_hi, :],
                        in1=V3[:, lo - 1:j_hi - 1, 33:65])
                lo = max(2, j_lo)
                if j_hi > lo:           # r=0: += v[:, j-2, 65]
                    eng.tensor_add(
                        out=Y3[:, lo:j_hi, 0], in0=Y3[:, lo:j_hi, 0],
                        in1=V3[:, lo - 2:j_hi - 2, 65])
                hi = min(j_hi, 31)
                if hi > j_lo:           # r=31: += v[:, j+1, 0]
                    eng.tensor_add(
                        out=Y3[:, j_lo:hi, 31], in0=Y3[:, j_lo:hi, 31],
                        in1=V3[:, j_lo + 1:hi + 1, 0])

            # rounds: after vtrans(3) -> j<15 (t<=15 avail); after vtrans(6)
            # -> j<27; after vtrans(7) -> all (j<33)
            asm_rounds = {3: (0, 15), 6: (15, 27), 7: (27, 33)}
            y_watermark = [0]

            prev = None          # (tcix, v_tiles) of the chunk awaiting fused stage
            xt_lists = {0: emit_loads_transposes(0)}
            emit_w1_loads()
            emit_const_loads()
            for tcix in range(TC):
                ht_tiles[tcix] = {
                    (c, hc): ht_pool.tile(
                        [128, TL * 128], FH if x16 else FR,
                        tag=f"ht{hc}_{c}", name=f"ht_{tcix}_{hc}_{c}")
                    for c in range(C) for hc in range(HC)}
                if tcix + 1 < TC:
                    xt_lists[tcix + 1] = emit_loads_transposes(tcix + 1)
                xt_list = xt_lists[tcix]
                # interleave: MLP1(tcix, c) with fused(tcix-1, c) so PE always
                # has matmul work while relu copies drain PSUM
                for c in range(C):
                    emit_mlp1(tcix, xt_list, c)
                    if prev is not None:
                        emit_fused(prev[0], prev[1], c)
                if prev is not None:
                    emit_vtrans(prev[0], prev[1])
                    if prev[0] in asm_rounds:
                        emit_y_assembly(*asm_rounds[prev[0]])
                del xt_lists[tcix]
                v_tiles = [
                    (lo, hi, var, vps_pool.tile([66, 512], FD, tag="v_ps", name=f"v_ps_{tcix}_{lo}"))
                    for (lo, hi, var) in chunk_ranges(tcix)]
                prev = (tcix, v_tiles)
            for c in range(C):
                emit_fused(prev[0], prev[1], c)
            emit_vtrans(prev[0], prev[1])
            emit_y_assembly(*asm_rounds[TC - 1])
            if debug:
                nc.sync.dma_start(out=dbg_v[:], in_=V_sb[:])

            # first half can ship as soon as blocks j<16 are final (round 2
            # writes from j=15 up, so emit both stores at the end; the split
            # still lets the first store overlap the final assembly)
            nc.sync.dma_start(out=y[:, 0:480], in_=y_sb[:, 0:480])
            nc.sync.dma_start(out=y[:, 480:SIG_LEN], in_=y_sb[:, 480:SIG_LEN])

    nc.finalize()
    return nc


def make_in_maps(inputs, x16=X16):
    """Per-core input maps (shared by kernel(), sim checks, and bench)."""
    x = np.asarray(inputs["encoder_output"], dtype=np.float32)
    W1 = np.asarray(inputs["W1"], np.float32)
    b1 = np.asarray(inputs["b1"], np.float32)

    G, Bvec = _host_prep(
        inputs["W1"], inputs["b1"], inputs["W2"], inputs["b2"],
        inputs["Winv"], inputs["binv"], inputs["Wconv"], inputs["bconv"])

    # pack G -> [128, HC*C*3*66]: g_sb[p, _g_col(hc,c,var)+m] = G[var, c, hc*128+p, m]
    g_pack = np.zeros((128, HC * C * 3 * 66), np.float32)
    for hc in range(HC):
        for c in range(C):
            for var in range(3):
                col = _g_col(hc, c, var)
                g_pack[:, col:col + 66] = G[var, c, hc * 128:(hc + 1) * 128, :]

    w1t = np.ascontiguousarray(W1.T)                        # [E, H]
    if x16:
        w1t = w1t.astype(np.float16)
        g_pack = g_pack.astype(np.float16)
    b1c = np.ascontiguousarray(b1.reshape(HC, 128).T)       # [128, HC]
    brep = np.ascontiguousarray(np.broadcast_to(Bvec, (BL, SIG_LEN)))
    ident = np.eye(128, dtype=np.float32)

    if x16:
        # [B,C,T,E] -> per-shard [C,T,BL,E] fp16
        xs = x.reshape(N_CORES, BL, C, T, E).transpose(0, 2, 3, 1, 4)
        xs = np.ascontiguousarray(xs.astype(np.float16))
    else:
        xs = x.reshape(N_CORES, BL, C, T, E)
    return [
        {
            "x": np.ascontiguousarray(xs[i]),
            "w1t": w1t, "b1c": b1c, "g": g_pack,
            "brep": brep, "ident": ident,
        }
        for i in range(N_CORES)
    ]


def kernel(**inputs) -> np.ndarray:
    if "nc" not in _CACHE:
        _CACHE["nc"] = _build_bass()
    nc = _CACHE["nc"]

    in_maps = make_in_maps(inputs)
    res = run_bass_kernel_spmd(nc, in_maps, core_ids=list(range(N_CORES)))
    _CACHE["last_result"] = res
    y = np.concatenate([r["y"] for r in res.results], axis=0)   # [B, 1056]
    return y.reshape(B, 1, SIG_LEN).astype(np.float32)


if __name__ == "__main__":
    rng = np.random.default_rng(0)
    ins = {
        "encoder_output": rng.standard_normal((B, C, T, E), dtype=np.float32),
        "W1": rng.standard_normal((H, E), dtype=np.float32) / np.sqrt(E),
        "b1": rng.standard_normal((H,), dtype=np.float32) / np.sqrt(E),
        "W2": rng.standard_normal((E, H), dtype=np.float32) / np.sqrt(H),
        "b2": rng.standard_normal((E,), dtype=np.float32) / np.sqrt(H),
        "Winv": rng.standard_normal((SEG_LEN, E), dtype=np.float32) / np.sqrt(E),
        "binv": rng.standard_normal((SEG_LEN,), dtype=np.float32) / np.sqrt(E),
        "Wconv": rng.standard_normal((1, C, 3), dtype=np.float32) / np.sqrt(C * 3),
        "bconv": rng.standard_normal((1,), dtype=np.float32) / np.sqrt(C * 3),
    }
    out = kernel(**ins)
    print("kernel output", out.shape, out.dtype)



# revision 3
# speedup vs baseline: 1.3072x; 1.3072x over previous
"""Trainium2 Bass kernel for nn_Decoder (MLP -> inverse token embedding ->
overlap-add -> channel-merge conv), data-parallel over batch on 8 NeuronCores.

Self-contained: hardcodes shapes; host-side numpy folds everything after the
first Linear+ReLU into per-channel fused matrices G (W2 -> Winv -> overlap-add
normalization -> 3-tap channel conv), so the device pipeline is:

    xT[E,tok] (host pre-transposed, fp16) --matmul W1T--> h[Hc,tok] in PSUM
    --ACT/DVE relu+bias--> hT in SBUF --matmul G (accum over c,Hc)--> v[66,tok]
    --PE transpose--> vT[b,66] --strided adds (overlap-add)--> y[b,1056]

Sharding: batch 1024 -> 8 cores x 128.
"""

import numpy as np

import concourse.bacc as bacc
import concourse.mybir as mybir
from concourse.bass_utils import run_bass_kernel_spmd
from concourse.tile import TileContext

# problem shapes (hardcoded per contract)
B, C, T, E, H = 1024, 8, 32, 128, 256
SEG_LEN, SIG_LEN, NUM_SEG, STEP = 64, 1056, 32, 32
N_CORES = 8
BL = B // N_CORES          # local batch per core = 128
HC = H // 128              # H chunks = 2
TC = 8                     # t-chunks
TL = T // TC               # t per chunk = 4
CW = TL * 128              # columns per (c, t-chunk) = 512
FD = mybir.dt.float32
FR = mybir.dt.float32r   # fp32 storage, FP22 multiply: 4x faster PE
FH = mybir.dt.float16

_CACHE = {}


def _host_prep(W1, b1, W2, b2, Winv, binv, Wconv, bconv):
    """Fold W2/Winv/normalization/conv into G [3var][C][H,66] and bias B[1056]."""
    counter = np.zeros(SIG_LEN, np.float64)
    for t in range(NUM_SEG):
        counter[t * STEP: t * STEP + SEG_LEN] += 1.0
    n = 1.0 / counter

    F = Winv.astype(np.float64) @ W2.astype(np.float64)          # [64, H]
    binv2 = Winv.astype(np.float64) @ b2.astype(np.float64) + binv.astype(np.float64)
    Wc = Wconv[0].astype(np.float64)                             # [C, 3]

    def n_of(var, s):
        if var == 0:
            return n[s]
        if var == 2:
            return n[992 + s]
        return 0.5

    G = np.zeros((3, C, H, 66), np.float64)
    for var in range(3):
        for c in range(C):
            for m_idx in range(66):
                for k in range(3):
                    s = m_idx + k - 2
                    if 0 <= s < SEG_LEN:
                        G[var, c, :, m_idx] += Wc[c, k] * n_of(var, s) * F[s, :]

    sig_b = np.zeros(SIG_LEN, np.float64)
    for t in range(NUM_SEG):
        sig_b[t * STEP: t * STEP + SEG_LEN] += binv2
    sig_b *= n
    Bvec = np.full(SIG_LEN, float(np.asarray(bconv).reshape(-1)[0]), np.float64)
    q = np.arange(SIG_LEN)
    for k in range(3):
        qq = q + k - 1
        valid = (qq >= 0) & (qq < SIG_LEN)
        for c in range(C):
            Bvec[valid] += Wc[c, k] * sig_b[qq[valid]]
    return G.astype(np.float32), Bvec.astype(np.float32)


def _g_col(hc, c, var):
    """Column offset of G slice (hc, c, var) inside g_sb [128, 2*8*3*66]."""
    return ((hc * C + c) * 3 + var) * 66


def _build_bass():
    nc = bacc.Bacc("TRN2")

    # host pre-transposed to [TC, E, C*TL*BL] fp16: per t-chunk one contiguous
    # [128, 4096] block, columns ordered (c, tl, b)
    x = nc.dram_tensor("x", [TC, E, C * CW], FH, kind="ExternalInput")
    w1t = nc.dram_tensor("w1t", [E, H], FH, kind="ExternalInput")
    b1c = nc.dram_tensor("b1c", [128, HC], FD, kind="ExternalInput")
    g = nc.dram_tensor("g", [128, HC * C * 3 * 66], FH, kind="ExternalInput")
    brep = nc.dram_tensor("brep", [BL, SIG_LEN], FD, kind="ExternalInput")
    ident = nc.dram_tensor("ident", [128, 128], FR, kind="ExternalInput")
    y = nc.dram_tensor("y", [BL, SIG_LEN], FD, kind="ExternalOutput")

    with TileContext(nc) as tc:
        with (
            tc.tile_pool(name="consts", bufs=1) as consts,
            tc.tile_pool(name="xin", bufs=3) as xin_pool,
            tc.tile_pool(name="ht", bufs=2) as ht_pool,
            tc.tile_pool(name="vsb", bufs=3) as vsb_pool,
            tc.tile_pool(name="big", bufs=1) as big_pool,
            tc.tile_pool(name="pe_out", bufs=1, space="PSUM") as peout_pool,
            tc.tile_pool(name="h_ps", bufs=4, space="PSUM") as hps_pool,
            tc.tile_pool(name="v_ps", bufs=3, space="PSUM") as vps_pool,
        ):
            w1t_sb = consts.tile([E, H], FH)
            b1c_sb = consts.tile([128, HC], FD)
            g_sb = consts.tile([128, HC * C * 3 * 66], FH)
            ident_sb = consts.tile([128, 128], FR)
            brep_sb = big_pool.tile([BL, SIG_LEN], FD)

            V_sb = big_pool.tile([BL, T * 66], FD)      # v transposed: [b, t*66+m]
            y_sb = big_pool.tile([BL, SIG_LEN], FD)

            xin_tiles = {}

            def emit_x_load(tcix):
                xt = xin_pool.tile([E, C * CW], FH, tag="xin")
                nc.sync.dma_start(out=xt[:], in_=x[tcix, :, :])
                xin_tiles[tcix] = xt

            def emit_const_loads():
                nc.sync.dma_start(out=w1t_sb[:], in_=w1t[:])
                nc.sync.dma_start(out=b1c_sb[:], in_=b1c[:])
                nc.sync.dma_start(out=g_sb[:], in_=g[:])
                nc.sync.dma_start(out=ident_sb[:], in_=ident[:])
                nc.sync.dma_start(out=brep_sb[:], in_=brep[:])

            # software pipeline: fused stage runs one t-chunk behind MLP1
            ht_tiles = {}

            # greedy ACT/DVE load balancer for PSUM->SBUF copies and relus
            eng_busy = {"act": 0.0, "dve": 0.0}

            def pick_engine(fd):
                ca = (172 + fd) / 0.96
                cd = (120 + fd) / 0.82
                if eng_busy["act"] + ca <= eng_busy["dve"] + cd:
                    eng_busy["act"] += ca
                    return "act"
                eng_busy["dve"] += cd
                return "dve"

            def bal_copy(out, in_, fd):
                if pick_engine(fd) == "act":
                    nc.scalar.copy(out=out, in_=in_)
                else:
                    nc.vector.tensor_copy(out=out, in_=in_)

            def chunk_ranges(tcix):
                # column ranges with uniform G variant; cols = tl*128 + b
                if tcix == 0:
                    return [(0, 128, 0), (128, CW, 1)]        # t=0 -> var 0
                if tcix == TC - 1:
                    return [(0, 384, 1), (384, CW, 2)]        # t=31 -> var 2
                return [(0, CW, 1)]

            def emit_mlp1(tcix, c):
                ht = ht_tiles[tcix]
                xt = xin_tiles[tcix]
                h_list = []
                for hc in range(HC):
                    h_ps = hps_pool.tile([128, CW], FD, tag="h_ps",
                                         name=f"h_ps_{tcix}_{c}_{hc}")
                    nc.tensor.matmul(
                        h_ps[:],
                        w1t_sb[:, hc * 128:(hc + 1) * 128],
                        xt[:, c * CW:(c + 1) * CW],
                        start=True, stop=True,
                    )
                    h_list.append(h_ps)
                for hc in range(HC):
                    # relu + bias -> hT slice; alternate ACT/DVE engines
                    dst = ht[(c, hc)][:]
                    src = h_list[hc][:]
                    if pick_engine(CW) == "act":
                        nc.scalar.activation(
                            dst, src,
                            mybir.ActivationFunctionType.Relu,
                            bias=b1c_sb[:, hc:hc + 1], scale=1.0,
                        )
                    else:
                        nc.vector.tensor_scalar(
                            dst, src,
                            b1c_sb[:, hc:hc + 1], 0.0,
                            mybir.AluOpType.add, mybir.AluOpType.max,
                        )

            def emit_fused(tcix, v_tiles, c):
                """fused G matmuls for channel c accumulating into v_tiles."""
                ht = ht_tiles[tcix]
                for (lo, hi, var, v_ps) in v_tiles:
                    for hc in range(HC):
                        i = c * HC + hc
                        nc.tensor.matmul(
                            v_ps[:, lo:hi],
                            g_sb[:, _g_col(hc, c, var):_g_col(hc, c, var) + 66],
                            ht[(c, hc)][:, lo:hi],
                            start=(i == 0), stop=(i == C * HC - 1),
                        )

            def emit_vtrans(tcix, v_tiles):
                """copy v psum -> sbuf, PE-transpose per t into V_sb."""
                del ht_tiles[tcix]
                v_sb = vsb_pool.tile([66, CW], FR, tag="v_sb")
                for (lo, hi, var, v_ps) in v_tiles:
                    bal_copy(v_sb[:, lo:hi], v_ps[:, lo:hi], hi - lo)
                for tl in range(TL):
                    t = tcix * TL + tl
                    vt_ps = peout_pool.tile([128, 66], FR, tag="pe_out")
                    nc.tensor.transpose(
                        vt_ps[:],
                        v_sb[:, tl * 128:(tl + 1) * 128],
                        ident_sb[0:66, 0:66],
                    )
                    bal_copy(V_sb[:, t * 66:(t + 1) * 66], vt_ps[:], 66)

            # overlap-add assembly in rounds (per watermark) so it overlaps
            # with later chunks instead of serializing at the end
            V3 = V_sb[:].rearrange("b (t m) -> b t m", m=66)
            Y3 = y_sb[:].rearrange("b (j r) -> b j r", r=32)
            B3 = brep_sb[:].rearrange("b (j r) -> b j r", r=32)

            def emit_y_assembly(j_lo, j_hi):
                """Assemble y blocks j in [j_lo, j_hi); requires V[t] for
                t <= j_hi (uses t=j+1 for the r=31 edge). Runs on GpSimd
                (SBUF-only) to keep DVE/ACT free for PSUM drains."""
                eng = nc.gpsimd
                jm = min(j_hi, 32)      # main1 defined for j<=31
                if jm > j_lo:
                    eng.tensor_add(
                        out=Y3[:, j_lo:jm, :], in0=V3[:, j_lo:jm, 1:33],
                        in1=B3[:, j_lo:jm, :])
                if j_hi == 33:          # last block: bias only here
                    eng.tensor_copy(
                        out=y_sb[:, 1024:1056], in_=brep_sb[:, 1024:1056])
                lo = max(1, j_lo)
                if j_hi > lo:           # += v[:, j-1, r+33]
                    eng.tensor_add(
                        out=Y3[:, lo:j_hi, :], in0=Y3[:, lo:j_hi, :],
                        in1=V3[:, lo - 1:j_hi - 1, 33:65])
                lo = max(2, j_lo)
                if j_hi > lo:           # r=0: += v[:, j-2, 65]
                    eng.tensor_add(
                        out=Y3[:, lo:j_hi, 0], in0=Y3[:, lo:j_hi, 0],
                        in1=V3[:, lo - 2:j_hi - 2, 65])
                hi = min(j_hi, 31)
                if hi > j_lo:           # r=31: += v[:, j+1, 0]
                    eng.tensor_add(
                        out=Y3[:, j_lo:hi, 31], in0=Y3[:, j_lo:hi, 31],
                        in1=V3[:, j_lo + 1:hi + 1, 0])

            # after vtrans(i) V[t] is final for t <= 4i+3, so y blocks
            # j < min(4i+3, 33) can assemble (block j reads up to t=j+1)
            y_wm = [0]

            def emit_rounds(i):
                j_hi = min(4 * i + 3, 33) if i < TC - 1 else 33
                if j_hi > y_wm[0]:
                    emit_y_assembly(y_wm[0], j_hi)
                    y_wm[0] = j_hi

            prev = None          # (tcix, v_tiles) of the chunk awaiting fused
            emit_x_load(0)
            emit_x_load(1)
            emit_const_loads()
            for tcix in range(TC):
                ht_tiles[tcix] = {
                    (c, hc): ht_pool.tile(
                        [128, CW], FH,
                        tag=f"ht{hc}_{c}", name=f"ht_{tcix}_{hc}_{c}")
                    for c in range(C) for hc in range(HC)}
                if tcix + 2 < TC:
                    emit_x_load(tcix + 2)
                # interleave: MLP1(tcix, c) with fused(tcix-1, c) so PE always
                # has matmul work while relu copies drain PSUM
                for c in range(C):
                    emit_mlp1(tcix, c)
                    if prev is not None:
                        emit_fused(prev[0], prev[1], c)
                if prev is not None:
                    emit_vtrans(prev[0], prev[1])
                    emit_rounds(prev[0])
                del xin_tiles[tcix]
                v_tiles = [
                    (lo, hi, var, vps_pool.tile(
                        [66, CW], FD, tag="v_ps", name=f"v_ps_{tcix}_{lo}"))
                    for (lo, hi, var) in chunk_ranges(tcix)]
                prev = (tcix, v_tiles)
            for c in range(C):
                emit_fused(prev[0], prev[1], c)
            emit_vtrans(prev[0], prev[1])
            emit_rounds(prev[0])

            # split stores so the first can overlap the final assembly round
            nc.sync.dma_start(out=y[:, 0:480], in_=y_sb[:, 0:480])
            nc.sync.dma_start(out=y[:, 480:SIG_LEN], in_=y_sb[:, 480:SIG_LEN])

    nc.finalize()
    return nc


def make_in_maps(inputs):
    """Per-core input maps (shared by kernel(), sim checks, and bench)."""
    x = np.asarray(inputs["encoder_output"], dtype=np.float32)
    W1 = np.asarray(inputs["W1"], np.float32)
    b1 = np.asarray(inputs["b1"], np.float32)

    G, Bvec = _host_prep(
        inputs["W1"], inputs["b1"], inputs["W2"], inputs["b2"],
        inputs["Winv"], inputs["binv"], inputs["Wconv"], inputs["bconv"])

    # pack G -> [128, HC*C*3*66]: g_sb[p, _g_col(hc,c,var)+m] = G[var, c, hc*128+p, m]
    g_pack = np.zeros((128, HC * C * 3 * 66), np.float32)
    for hc in range(HC):
        for c in range(C):
            for var in range(3):
                col = _g_col(hc, c, var)
                g_pack[:, col:col + 66] = G[var, c, hc * 128:(hc + 1) * 128, :]

    w1t = np.ascontiguousarray(W1.T).astype(np.float16)     # [E, H]
    g_pack = g_pack.astype(np.float16)
    b1c = np.ascontiguousarray(b1.reshape(HC, 128).T)       # [128, HC]
    brep = np.ascontiguousarray(np.broadcast_to(Bvec, (BL, SIG_LEN)))
    ident = np.eye(128, dtype=np.float32)

    # [B,C,T,E] -> per-core [TC, E, (c, tl, b)] fp16 (device reads xT directly)
    xs = x.reshape(N_CORES, BL, C, TC, TL, E).transpose(0, 3, 5, 2, 4, 1)
    xs = np.ascontiguousarray(xs.astype(np.float16)).reshape(
        N_CORES, TC, E, C * CW)
    return [
        {
            "x": xs[i],
            "w1t": w1t, "b1c": b1c, "g": g_pack,
            "brep": brep, "ident": ident,
        }
        for i in range(N_CORES)
    ]


def kernel(**inputs) -> np.ndarray:
    if "nc" not in _CACHE:
        _CACHE["nc"] = _build_bass()
    nc = _CACHE["nc"]

    in_maps = make_in_maps(inputs)
    res = run_bass_kernel_spmd(nc, in_maps, core_ids=list(range(N_CORES)))
    _CACHE["last_result"] = res
    y = np.concatenate([r["y"] for r in res.results], axis=0)   # [B, 1056]
    return y.reshape(B, 1, SIG_LEN).astype(np.float32)


if __name__ == "__main__":
    rng = np.random.default_rng(0)
    ins = {
        "encoder_output": rng.standard_normal((B, C, T, E), dtype=np.float32),
        "W1": rng.standard_normal((H, E), dtype=np.float32) / np.sqrt(E),
        "b1": rng.standard_normal((H,), dtype=np.float32) / np.sqrt(E),
        "W2": rng.standard_normal((E, H), dtype=np.float32) / np.sqrt(H),
        "b2": rng.standard_normal((E,), dtype=np.float32) / np.sqrt(H),
        "Winv": rng.standard_normal((SEG_LEN, E), dtype=np.float32) / np.sqrt(E),
        "binv": rng.standard_normal((SEG_LEN,), dtype=np.float32) / np.sqrt(E),
        "Wconv": rng.standard_normal((1, C, 3), dtype=np.float32) / np.sqrt(C * 3),
        "bconv": rng.standard_normal((1,), dtype=np.float32) / np.sqrt(C * 3),
    }
    out = kernel(**ins)
    print("kernel output", out.shape, out.dtype)


# revision 8
# speedup vs baseline: 1.4913x; 1.1408x over previous
"""Trainium2 Bass kernel for nn_Decoder (MLP -> inverse token embedding ->
overlap-add -> channel-merge conv), data-parallel over batch on 8 NeuronCores.

Self-contained: hardcodes shapes; host-side numpy folds everything after the
first Linear+ReLU into per-channel fused matrices G (W2 -> Winv -> overlap-add
normalization -> 3-tap channel conv), so the device pipeline is:

    xT[E,tok] (host pre-transposed, fp16) --matmul W1T--> h[Hc,tok] in PSUM
    --ACT/DVE relu+bias--> hT in SBUF --matmul G (accum over c,Hc)--> v[66,tok]
    --PE transpose--> vT[b,66] --strided adds (overlap-add)--> y[b,1056]

Sharding: batch 1024 -> 8 cores x 128.
"""

import numpy as np

import concourse.bacc as bacc
import concourse.mybir as mybir
from concourse.bass_utils import run_bass_kernel_spmd
from concourse.tile import TileContext

# problem shapes (hardcoded per contract)
B, C, T, E, H = 1024, 8, 32, 128, 256
SEG_LEN, SIG_LEN, NUM_SEG, STEP = 64, 1056, 32, 32
N_CORES = 8
BL = B // N_CORES          # local batch per core = 128
HC = H // 128              # H chunks = 2
TC = 8                     # t-chunks
TL = T // TC               # t per chunk = 4
CW = TL * 128              # columns per (c, t-chunk) = 512
FD = mybir.dt.float32
FR = mybir.dt.float32r   # fp32 storage, FP22 multiply: 4x faster PE
FH = mybir.dt.float16

_CACHE = {}


def _host_prep(W1, b1, W2, b2, Winv, binv, Wconv, bconv):
    """Fold W2/Winv/normalization/conv into G [3var][C][H,66] and bias B[1056]."""
    counter = np.zeros(SIG_LEN, np.float64)
    for t in range(NUM_SEG):
        counter[t * STEP: t * STEP + SEG_LEN] += 1.0
    n = 1.0 / counter

    F = Winv.astype(np.float64) @ W2.astype(np.float64)          # [64, H]
    binv2 = Winv.astype(np.float64) @ b2.astype(np.float64) + binv.astype(np.float64)
    Wc = Wconv[0].astype(np.float64)                             # [C, 3]

    def n_of(var, s):
        if var == 0:
            return n[s]
        if var == 2:
            return n[992 + s]
        return 0.5

    G = np.zeros((3, C, H, 66), np.float64)
    for var in range(3):
        for c in range(C):
            for m_idx in range(66):
                for k in range(3):
                    s = m_idx + k - 2
                    if 0 <= s < SEG_LEN:
                        G[var, c, :, m_idx] += Wc[c, k] * n_of(var, s) * F[s, :]

    sig_b = np.zeros(SIG_LEN, np.float64)
    for t in range(NUM_SEG):
        sig_b[t * STEP: t * STEP + SEG_LEN] += binv2
    sig_b *= n
    Bvec = np.full(SIG_LEN, float(np.asarray(bconv).reshape(-1)[0]), np.float64)
    q = np.arange(SIG_LEN)
    for k in range(3):
        qq = q + k - 1
        valid = (qq >= 0) & (qq < SIG_LEN)
        for c in range(C):
            Bvec[valid] += Wc[c, k] * sig_b[qq[valid]]
    return G.astype(np.float32), Bvec.astype(np.float32)


def _g_col(hc, c, var):
    """Column offset of G slice (hc, c, var) inside g_sb [128, 2*8*3*66]."""
    return ((hc * C + c) * 3 + var) * 66


def _build_bass():
    nc = bacc.Bacc("TRN2")

    # host pre-transposed to [TC, E, C*TL*BL] fp16: per t-chunk one contiguous
    # [128, 4096] block, columns ordered (c, tl, b)
    x = nc.dram_tensor("x", [TC, E, C * CW], FH, kind="ExternalInput")
    w1t = nc.dram_tensor("w1t", [E, H], FH, kind="ExternalInput")
    b1c = nc.dram_tensor("b1c", [128, HC], FD, kind="ExternalInput")
    g = nc.dram_tensor("g", [128, HC * C * 3 * 66], FH, kind="ExternalInput")
    brep = nc.dram_tensor("brep", [BL, SIG_LEN], FD, kind="ExternalInput")
    ident = nc.dram_tensor("ident", [128, 128], FR, kind="ExternalInput")
    y = nc.dram_tensor("y", [BL, SIG_LEN], FD, kind="ExternalOutput")

    with TileContext(nc) as tc:
        with (
            tc.tile_pool(name="consts", bufs=1) as consts,
            tc.tile_pool(name="xin", bufs=3) as xin_pool,
            tc.tile_pool(name="ht", bufs=2) as ht_pool,
            tc.tile_pool(name="vsb", bufs=3) as vsb_pool,
            tc.tile_pool(name="big", bufs=1) as big_pool,
            tc.tile_pool(name="pe_out", bufs=1, space="PSUM") as peout_pool,
            tc.tile_pool(name="h_ps", bufs=2, space="PSUM") as hps_pool,
            tc.tile_pool(name="v_ps", bufs=3, space="PSUM") as vps_pool,
        ):
            w1t_sb = consts.tile([E, H], FH)
            b1c_sb = consts.tile([128, HC], FD)
            g_sb = consts.tile([128, HC * C * 3 * 66], FH)
            ident_sb = consts.tile([128, 128], FR)
            warm_a = consts.tile([128, 128], FH)
            warm_b = consts.tile([128, 256], FH)
            brep_sb = big_pool.tile([BL, SIG_LEN], FD)

            V_sb = big_pool.tile([BL, T * 66], FD)      # v transposed: [b, t*66+m]
            y_sb = big_pool.tile([BL, SIG_LEN], FD)

            xin_tiles = {}

            def emit_x_load(tcix, split_first=False):
                xt = xin_pool.tile([E, C * CW], FH, tag="xin")
                if split_first:
                    # first chunk: land c0-c1 early so MLP1 starts sooner
                    nc.sync.dma_start(out=xt[:, 0:2 * CW],
                                      in_=x[tcix, :, 0:2 * CW])
                    xin_tiles[tcix] = (xt, True)
                    return
                nc.sync.dma_start(out=xt[:], in_=x[tcix, :, :])
                xin_tiles[tcix] = (xt, False)

            def emit_x_load_rest(tcix):
                xt, _ = xin_tiles[tcix]
                nc.sync.dma_start(out=xt[:, 2 * CW:C * CW],
                                  in_=x[tcix, :, 2 * CW:C * CW])

            # PE warm-up: the HAM clock gate releases only after ~3.4us of
            # sustained PE activity; burn garbage matmuls during the DMA head
            # so real matmuls start at 2.4 GHz. Reuses an h_ps buffer (WAW on
            # the in-order PE queue, so no stall).
            def emit_warmup():
                warm_ps = hps_pool.tile([128, 2 * CW], FD, tag="h_ps",
                                        name="warm_ps")
                nc.gpsimd.memset(warm_a[:], 1.0)
                nc.gpsimd.memset(warm_b[:], 1.0)
                for _ in range(16):
                    nc.tensor.matmul(
                        warm_ps[:, 0:256], warm_a[:], warm_b[:],
                        start=True, stop=True)

            # software pipeline: fused stage runs one t-chunk behind MLP1
            ht_tiles = {}

            # greedy ACT/DVE load balancer for PSUM->SBUF copies and relus
            eng_busy = {"act": 0.0, "dve": 0.0}

            def pick_engine(fd):
                ca = (352 + fd) / 1.2
                cd = (151 + fd) / 0.96
                if eng_busy["act"] + ca <= eng_busy["dve"] + cd:
                    eng_busy["act"] += ca
                    return "act"
                eng_busy["dve"] += cd
                return "dve"

            def bal_copy(out, in_, fd):
                if pick_engine(fd) == "act":
                    nc.scalar.copy(out=out, in_=in_)
                else:
                    nc.vector.tensor_copy(out=out, in_=in_)

            def chunk_ranges(tcix):
                # column ranges with uniform G variant; cols = tl*128 + b
                if tcix == 0:
                    return [(0, 128, 0), (128, CW, 1)]        # t=0 -> var 0
                if tcix == TC - 1:
                    return [(0, 384, 1), (384, CW, 2)]        # t=31 -> var 2
                return [(0, CW, 1)]

            def emit_mlp1_pair(tcix, cp):
                """MLP1 for channel pair (2cp, 2cp+1): 2 matmuls per hc into
                one 2-bank PSUM tile, drained by a single 1024-wide relu."""
                ht = ht_tiles[tcix]
                xt, _ = xin_tiles[tcix]
                h_list = []
                for hc in range(HC):
                    h_ps = hps_pool.tile([128, 2 * CW], FD, tag="h_ps",
                                         name=f"h_ps_{tcix}_{cp}_{hc}")
                    for half in range(2):
                        c = 2 * cp + half
                        nc.tensor.matmul(
                            h_ps[:, half * CW:(half + 1) * CW],
                            w1t_sb[:, hc * 128:(hc + 1) * 128],
                            xt[:, c * CW:(c + 1) * CW],
                            start=True, stop=True,
                        )
                    h_list.append(h_ps)
                for hc in range(HC):
                    dst = ht[(cp, hc)][:]
                    src = h_list[hc][:]
                    if pick_engine(2 * CW) == "act":
                        nc.scalar.activation(
                            dst, src,
                            mybir.ActivationFunctionType.Relu,
                            bias=b1c_sb[:, hc:hc + 1], scale=1.0,
                        )
                    else:
                        nc.vector.tensor_scalar(
                            dst, src,
                            b1c_sb[:, hc:hc + 1], 0.0,
                            mybir.AluOpType.add, mybir.AluOpType.max,
                        )

            def emit_fused(tcix, v_tiles, c):
                """fused G matmuls for channel c accumulating into v_tiles."""
                ht = ht_tiles[tcix]
                cp, half = divmod(c, 2)
                for (lo, hi, var, v_ps) in v_tiles:
                    for hc in range(HC):
                        i = c * HC + hc
                        nc.tensor.matmul(
                            v_ps[:, lo:hi],
                            g_sb[:, _g_col(hc, c, var):_g_col(hc, c, var) + 66],
                            ht[(cp, hc)][:, half * CW + lo:half * CW + hi],
                            start=(i == 0), stop=(i == C * HC - 1),
                        )

            def emit_vtrans(tcix, v_tiles):
                """copy v psum -> sbuf, PE-transpose per t into one PSUM tile,
                single merged copy into V_sb."""
                del ht_tiles[tcix]
                v_sb = vsb_pool.tile([66, CW], FR, tag="v_sb")
                for (lo, hi, var, v_ps) in v_tiles:
                    bal_copy(v_sb[:, lo:hi], v_ps[:, lo:hi], hi - lo)
                vt_ps = peout_pool.tile([128, 264], FR, tag="pe_out")
                for tl in range(TL):
                    nc.tensor.transpose(
                        vt_ps[:, tl * 66:(tl + 1) * 66],
                        v_sb[:, tl * 128:(tl + 1) * 128],
                        ident_sb[0:66, 0:66],
                    )
                bal_copy(V_sb[:, tcix * 264:(tcix + 1) * 264], vt_ps[:], 264)

            # overlap-add assembly in rounds (per watermark) so it overlaps
            # with later chunks instead of serializing at the end
            V3 = V_sb[:].rearrange("b (t m) -> b t m", m=66)
            Y3 = y_sb[:].rearrange("b (j r) -> b j r", r=32)
            B3 = brep_sb[:].rearrange("b (j r) -> b j r", r=32)

            def emit_y_assembly(j_lo, j_hi, eng):
                """Assemble y blocks j in [j_lo, j_hi); requires V[t] for
                t <= j_hi (uses t=j+1 for the r=31 edge)."""
                jm = min(j_hi, 32)      # main1 defined for j<=31
                if jm > j_lo:
                    eng.tensor_add(
                        out=Y3[:, j_lo:jm, :], in0=V3[:, j_lo:jm, 1:33],
                        in1=B3[:, j_lo:jm, :])
                if j_hi == 33:          # last block: bias only here
                    eng.tensor_copy(
                        out=y_sb[:, 1024:1056], in_=brep_sb[:, 1024:1056])
                lo = max(1, j_lo)
                if j_hi > lo:           # += v[:, j-1, r+33]
                    eng.tensor_add(
                        out=Y3[:, lo:j_hi, :], in0=Y3[:, lo:j_hi, :],
                        in1=V3[:, lo - 1:j_hi - 1, 33:65])
                lo = max(2, j_lo)
                if j_hi > lo:           # r=0: += v[:, j-2, 65]
                    eng.tensor_add(
                        out=Y3[:, lo:j_hi, 0], in0=Y3[:, lo:j_hi, 0],
                        in1=V3[:, lo - 2:j_hi - 2, 65])
                hi = min(j_hi, 31)
                if hi > j_lo:           # r=31: += v[:, j+1, 0]
                    eng.tensor_add(
                        out=Y3[:, j_lo:hi, 31], in0=Y3[:, j_lo:hi, 31],
                        in1=V3[:, j_lo + 1:hi + 1, 0])

            # after vtrans(i) V[t] is final for t <= 4i+3, so y blocks
            # j < min(4i+3, 33) can assemble (block j reads up to t=j+1)
            y_wm = [0]

            def emit_rounds(i):
                if i < TC - 1:
                    j_hi = min(4 * i + 3, 33)
                    if j_hi > y_wm[0]:
                        emit_y_assembly(y_wm[0], j_hi, nc.gpsimd)
                        y_wm[0] = j_hi
                else:
                    # final round: split across gpsimd + vector (independent
                    # j ranges) to shorten the tail
                    mid = (y_wm[0] + 33 + 1) // 2
                    emit_y_assembly(y_wm[0], mid, nc.gpsimd)
                    emit_y_assembly(mid, 33, nc.vector)
                    y_wm[0] = 33
                # progressive stores once column ranges are final
                if y_wm[0] >= 15 and not store_done[0]:
                    nc.sync.dma_start(out=y[:, 0:480], in_=y_sb[:, 0:480])
                    store_done[0] = True
                if y_wm[0] >= 27 and not store_done[1]:
                    nc.sync.dma_start(out=y[:, 480:864], in_=y_sb[:, 480:864])
                    store_done[1] = True

            store_done = [False, False]

            prev = None          # (tcix, v_tiles) of the chunk awaiting fused
            emit_warmup()
            emit_x_load(0, split_first=True)
            emit_const_loads_order = [
                (w1t_sb, w1t), (b1c_sb, b1c)]
            for sb, dr in emit_const_loads_order:
                nc.sync.dma_start(out=sb[:], in_=dr[:])
            emit_x_load_rest(0)
            emit_x_load(1)
            nc.sync.dma_start(out=g_sb[:], in_=g[:])
            nc.sync.dma_start(out=ident_sb[:], in_=ident[:])
            nc.sync.dma_start(out=brep_sb[:], in_=brep[:])
            for tcix in range(TC):
                ht_tiles[tcix] = {
                    (cp, hc): ht_pool.tile(
                        [128, 2 * CW], FH,
                        tag=f"ht{hc}_{cp}", name=f"ht_{tcix}_{hc}_{cp}")
                    for cp in range(C // 2) for hc in range(HC)}
                if tcix + 2 < TC:
                    emit_x_load(tcix + 2)
                # interleave: MLP1(tcix, cp) with fused(tcix-1, c) so PE
                # always has matmul work while relu copies drain PSUM
                for cp in range(C // 2):
                    emit_mlp1_pair(tcix, cp)
                    if prev is not None:
                        emit_fused(prev[0], prev[1], 2 * cp)
                        emit_fused(prev[0], prev[1], 2 * cp + 1)
                if prev is not None:
                    emit_vtrans(prev[0], prev[1])
                    emit_rounds(prev[0])
                del xin_tiles[tcix]
                v_tiles = [
                    (lo, hi, var, vps_pool.tile(
                        [66, CW], FD, tag="v_ps", name=f"v_ps_{tcix}_{lo}"))
                    for (lo, hi, var) in chunk_ranges(tcix)]
                prev = (tcix, v_tiles)
            for c in range(C):
                emit_fused(prev[0], prev[1], c)
            emit_vtrans(prev[0], prev[1])
            emit_rounds(prev[0])

            nc.sync.dma_start(out=y[:, 864:SIG_LEN], in_=y_sb[:, 864:SIG_LEN])

    nc.finalize()
    return nc


def make_in_maps(inputs):
    """Per-core input maps (shared by kernel(), sim checks, and bench)."""
    x = np.asarray(inputs["encoder_output"], dtype=np.float32)
    W1 = np.asarray(inputs["W1"], np.float32)
    b1 = np.asarray(inputs["b1"], np.float32)

    G, Bvec = _host_prep(
        inputs["W1"], inputs["b1"], inputs["W2"], inputs["b2"],
        inputs["Winv"], inputs["binv"], inputs["Wconv"], inputs["bconv"])

    # pack G -> [128, HC*C*3*66]: g_sb[p, _g_col(hc,c,var)+m] = G[var, c, hc*128+p, m]
    g_pack = np.zeros((128, HC * C * 3 * 66), np.float32)
    for hc in range(HC):
        for c in range(C):
            for var in range(3):
                col = _g_col(hc, c, var)
                g_pack[:, col:col + 66] = G[var, c, hc * 128:(hc + 1) * 128, :]

    w1t = np.ascontiguousarray(W1.T).astype(np.float16)     # [E, H]
    g_pack = g_pack.astype(np.float16)
    b1c = np.ascontiguousarray(b1.reshape(HC, 128).T)       # [128, HC]
    brep = np.ascontiguousarray(np.broadcast_to(Bvec, (BL, SIG_LEN)))
    ident = np.eye(128, dtype=np.float32)

    # [B,C,T,E] -> per-core [TC, E, (c, tl, b)] fp16 (device reads xT directly)
    xs = x.reshape(N_CORES, BL, C, TC, TL, E).transpose(0, 3, 5, 2, 4, 1)
    xs = np.ascontiguousarray(xs.astype(np.float16)).reshape(
        N_CORES, TC, E, C * CW)
    return [
        {
            "x": xs[i],
            "w1t": w1t, "b1c": b1c, "g": g_pack,
            "brep": brep, "ident": ident,
        }
        for i in range(N_CORES)
    ]


def kernel(**inputs) -> np.ndarray:
    if "nc" not in _CACHE:
        _CACHE["nc"] = _build_bass()
    nc = _CACHE["nc"]

    in_maps = make_in_maps(inputs)
    res = run_bass_kernel_spmd(nc, in_maps, core_ids=list(range(N_CORES)))
    _CACHE["last_result"] = res
    y = np.concatenate([r["y"] for r in res.results], axis=0)   # [B, 1056]
    return y.reshape(B, 1, SIG_LEN).astype(np.float32)


if __name__ == "__main__":
    rng = np.random.default_rng(0)
    ins = {
        "encoder_output": rng.standard_normal((B, C, T, E), dtype=np.float32),
        "W1": rng.standard_normal((H, E), dtype=np.float32) / np.sqrt(E),
        "b1": rng.standard_normal((H,), dtype=np.float32) / np.sqrt(E),
        "W2": rng.standard_normal((E, H), dtype=np.float32) / np.sqrt(H),
        "b2": rng.standard_normal((E,), dtype=np.float32) / np.sqrt(H),
        "Winv": rng.standard_normal((SEG_LEN, E), dtype=np.float32) / np.sqrt(E),
        "binv": rng.standard_normal((SEG_LEN,), dtype=np.float32) / np.sqrt(E),
        "Wconv": rng.standard_normal((1, C, 3), dtype=np.float32) / np.sqrt(C * 3),
        "bconv": rng.standard_normal((1,), dtype=np.float32) / np.sqrt(C * 3),
    }
    out = kernel(**ins)
    print("kernel output", out.shape, out.dtype)


# revision 14
# speedup vs baseline: 1.5177x; 1.0177x over previous
"""Trainium2 Bass kernel for nn_Decoder (MLP -> inverse token embedding ->
overlap-add -> channel-merge conv), data-parallel over batch on 8 NeuronCores.

Self-contained: hardcodes shapes; host-side numpy folds everything after the
first Linear+ReLU into per-channel fused matrices G (W2 -> Winv -> overlap-add
normalization -> 3-tap channel conv), so the device pipeline is:

    xT[E,tok] (host pre-transposed, fp16) --matmul W1T--> h[Hc,tok] in PSUM
    --ACT/DVE relu+bias--> hT in SBUF --matmul G (accum over c,Hc)--> v[66,tok]
    --PE transpose--> vT[b,66] --strided adds (overlap-add)--> y[b,1056]

Sharding: batch 1024 -> 8 cores x 128.
"""

import numpy as np

import concourse.bacc as bacc
import concourse.mybir as mybir
from concourse.bass_utils import run_bass_kernel_spmd
from concourse.tile import TileContext

# problem shapes (hardcoded per contract)
B, C, T, E, H = 1024, 8, 32, 128, 256
SEG_LEN, SIG_LEN, NUM_SEG, STEP = 64, 1056, 32, 32
N_CORES = 8
BL = B // N_CORES          # local batch per core = 128
HC = H // 128              # H chunks = 2
TC = 8                     # t-chunks
TL = T // TC               # t per chunk = 4
CW = TL * 128              # columns per (c, t-chunk) = 512
FD = mybir.dt.float32
FR = mybir.dt.float32r   # fp32 storage, FP22 multiply: 4x faster PE
FH = mybir.dt.float16

_CACHE = {}


def _host_prep(W1, b1, W2, b2, Winv, binv, Wconv, bconv):
    """Fold W2/Winv/normalization/conv into G [3var][C][H,66] and bias B[1056]."""
    counter = np.zeros(SIG_LEN, np.float64)
    for t in range(NUM_SEG):
        counter[t * STEP: t * STEP + SEG_LEN] += 1.0
    n = 1.0 / counter

    F = Winv.astype(np.float64) @ W2.astype(np.float64)          # [64, H]
    binv2 = Winv.astype(np.float64) @ b2.astype(np.float64) + binv.astype(np.float64)
    Wc = Wconv[0].astype(np.float64)                             # [C, 3]

    def n_of(var, s):
        if var == 0:
            return n[s]
        if var == 2:
            return n[992 + s]
        return 0.5

    G = np.zeros((3, C, H, 66), np.float64)
    for var in range(3):
        for c in range(C):
            for m_idx in range(66):
                for k in range(3):
                    s = m_idx + k - 2
                    if 0 <= s < SEG_LEN:
                        G[var, c, :, m_idx] += Wc[c, k] * n_of(var, s) * F[s, :]

    sig_b = np.zeros(SIG_LEN, np.float64)
    for t in range(NUM_SEG):
        sig_b[t * STEP: t * STEP + SEG_LEN] += binv2
    sig_b *= n
    Bvec = np.full(SIG_LEN, float(np.asarray(bconv).reshape(-1)[0]), np.float64)
    q = np.arange(SIG_LEN)
    for k in range(3):
        qq = q + k - 1
        valid = (qq >= 0) & (qq < SIG_LEN)
        for c in range(C):
            Bvec[valid] += Wc[c, k] * sig_b[qq[valid]]
    return G.astype(np.float32), Bvec.astype(np.float32)


def _g_col(hc, c, var):
    """Column offset of G slice (hc, c, var) inside g_sb [128, 2*8*3*66]."""
    return ((hc * C + c) * 3 + var) * 66


def _build_bass():
    nc = bacc.Bacc("TRN2")

    # host pre-transposed to [TC, E, C*TL*BL] fp16: per t-chunk one contiguous
    # [128, 4096] block, columns ordered (c, tl, b)
    x = nc.dram_tensor("x", [TC, E, C * CW], FH, kind="ExternalInput")
    w1t = nc.dram_tensor("w1t", [E, H], FH, kind="ExternalInput")
    b1c = nc.dram_tensor("b1c", [128, HC], FD, kind="ExternalInput")
    g = nc.dram_tensor("g", [128, HC * C * 3 * 66], FH, kind="ExternalInput")
    brep = nc.dram_tensor("brep", [BL, SIG_LEN], FD, kind="ExternalInput")
    ident = nc.dram_tensor("ident", [128, 128], FR, kind="ExternalInput")
    y = nc.dram_tensor("y", [BL, SIG_LEN], FD, kind="ExternalOutput")

    with TileContext(nc) as tc:
        with (
            tc.tile_pool(name="consts", bufs=1) as consts,
            tc.tile_pool(name="xin", bufs=3) as xin_pool,
            tc.tile_pool(name="ht", bufs=2) as ht_pool,
            tc.tile_pool(name="vsb", bufs=3) as vsb_pool,
            tc.tile_pool(name="big", bufs=1) as big_pool,
            tc.tile_pool(name="pe_out", bufs=1, space="PSUM") as peout_pool,
            tc.tile_pool(name="h_ps", bufs=2, space="PSUM") as hps_pool,
            tc.tile_pool(name="v_ps", bufs=3, space="PSUM") as vps_pool,
        ):
            w1t_sb = consts.tile([E, H], FH)
            b1c_sb = consts.tile([128, HC], FD)
            g_sb = consts.tile([128, HC * C * 3 * 66], FH)
            ident_sb = consts.tile([128, 128], FR)
            warm_a = consts.tile([128, 128], FH)
            warm_b = consts.tile([128, 256], FH)
            brep_sb = big_pool.tile([BL, SIG_LEN], FD)

            V_sb = big_pool.tile([BL, T * 66], FD)      # v transposed: [b, t*66+m]
            y_sb = big_pool.tile([BL, SIG_LEN], FD)

            xin_tiles = {}

            def emit_x_load(tcix, split_first=False):
                xt = xin_pool.tile([E, C * CW], FH, tag="xin")
                if split_first:
                    # first chunk: land c0-c1 early so MLP1 starts sooner
                    nc.sync.dma_start(out=xt[:, 0:2 * CW],
                                      in_=x[tcix, :, 0:2 * CW])
                    xin_tiles[tcix] = (xt, True)
                    return
                nc.sync.dma_start(out=xt[:], in_=x[tcix, :, :])
                xin_tiles[tcix] = (xt, False)

            def emit_x_load_rest(tcix):
                xt, _ = xin_tiles[tcix]
                nc.sync.dma_start(out=xt[:, 2 * CW:C * CW],
                                  in_=x[tcix, :, 2 * CW:C * CW])

            # PE warm-up: the HAM clock gate releases only after ~3.4us of
            # sustained PE activity; burn garbage matmuls during the DMA head
            # so real matmuls start at 2.4 GHz. Reuses an h_ps buffer (WAW on
            # the in-order PE queue, so no stall).
            def emit_warmup():
                warm_ps = hps_pool.tile([128, 2 * CW], FD, tag="h_ps",
                                        name="warm_ps")
                nc.gpsimd.memset(warm_a[:], 1.0)
                nc.gpsimd.memset(warm_b[:], 1.0)
                # trigger ACT_TABLE_LOAD now, not at the first real relu
                nc.scalar.activation(
                    warm_a[:], warm_a[:],
                    mybir.ActivationFunctionType.Relu, scale=1.0)
                for _ in range(16):
                    nc.tensor.matmul(
                        warm_ps[:, 0:256], warm_b[:, 0:128], warm_b[:],
                        start=True, stop=True)

            # software pipeline: fused stage runs one t-chunk behind MLP1
            ht_tiles = {}

            # greedy ACT/DVE load balancer for PSUM->SBUF copies and relus
            eng_busy = {"act": 0.0, "dve": 0.0}

            def pick_engine(fd):
                ca = (352 + fd) / 1.2
                cd = (151 + fd) / 0.96
                if eng_busy["act"] + ca <= eng_busy["dve"] + cd:
                    eng_busy["act"] += ca
                    return "act"
                eng_busy["dve"] += cd
                return "dve"

            def bal_copy(out, in_, fd):
                if pick_engine(fd) == "act":
                    nc.scalar.copy(out=out, in_=in_)
                else:
                    nc.vector.tensor_copy(out=out, in_=in_)

            def chunk_ranges(tcix):
                # column ranges with uniform G variant; cols = tl*128 + b
                if tcix == 0:
                    return [(0, 128, 0), (128, CW, 1)]        # t=0 -> var 0
                if tcix == TC - 1:
                    return [(0, 384, 1), (384, CW, 2)]        # t=31 -> var 2
                return [(0, CW, 1)]

            def emit_mlp1_pair(tcix, cp):
                """MLP1 for channel pair (2cp, 2cp+1): 2 matmuls per hc into
                one 2-bank PSUM tile, drained by a single 1024-wide relu."""
                ht = ht_tiles[tcix]
                xt, _ = xin_tiles[tcix]
                h_list = []
                for hc in range(HC):
                    h_ps = hps_pool.tile([128, 2 * CW], FD, tag="h_ps",
                                         name=f"h_ps_{tcix}_{cp}_{hc}")
                    # PSUM matmul out is capped at one bank (512 fp32), so
                    # two 512-col matmuls fill the 2-bank tile
                    for half in range(2):
                        c = 2 * cp + half
                        nc.tensor.matmul(
                            h_ps[:, half * CW:(half + 1) * CW],
                            w1t_sb[:, hc * 128:(hc + 1) * 128],
                            xt[:, c * CW:(c + 1) * CW],
                            start=True, stop=True,
                        )
                    h_list.append(h_ps)
                for hc in range(HC):
                    dst = ht[(cp, hc)][:]
                    src = h_list[hc][:]
                    if pick_engine(2 * CW) == "act":
                        nc.scalar.activation(
                            dst, src,
                            mybir.ActivationFunctionType.Relu,
                            bias=b1c_sb[:, hc:hc + 1], scale=1.0,
                        )
                    else:
                        nc.vector.tensor_scalar(
                            dst, src,
                            b1c_sb[:, hc:hc + 1], 0.0,
                            mybir.AluOpType.add, mybir.AluOpType.max,
                        )

            def emit_fused(tcix, v_tiles, c):
                """fused G matmuls for channel c accumulating into v_tiles."""
                ht = ht_tiles[tcix]
                cp, half = divmod(c, 2)
                for (lo, hi, var, v_ps) in v_tiles:
                    for hc in range(HC):
                        i = c * HC + hc
                        nc.tensor.matmul(
                            v_ps[:, lo:hi],
                            g_sb[:, _g_col(hc, c, var):_g_col(hc, c, var) + 66],
                            ht[(cp, hc)][:, half * CW + lo:half * CW + hi],
                            start=(i == 0), stop=(i == C * HC - 1),
                        )

            def emit_vtrans(tcix, v_tiles):
                """copy v psum -> sbuf, PE-transpose per t into one PSUM tile,
                single merged copy into V_sb."""
                del ht_tiles[tcix]
                v_sb = vsb_pool.tile([66, CW], FR, tag="v_sb")
                for (lo, hi, var, v_ps) in v_tiles:
                    bal_copy(v_sb[:, lo:hi], v_ps[:, lo:hi], hi - lo)
                vt_ps = peout_pool.tile([128, 264], FR, tag="pe_out")
                for tl in range(TL):
                    nc.tensor.transpose(
                        vt_ps[:, tl * 66:(tl + 1) * 66],
                        v_sb[:, tl * 128:(tl + 1) * 128],
                        ident_sb[0:66, 0:66],
                    )
                bal_copy(V_sb[:, tcix * 264:(tcix + 1) * 264], vt_ps[:], 264)

            # overlap-add assembly in rounds (per watermark) so it overlaps
            # with later chunks instead of serializing at the end
            V3 = V_sb[:].rearrange("b (t m) -> b t m", m=66)
            Y3 = y_sb[:].rearrange("b (j r) -> b j r", r=32)
            B3 = brep_sb[:].rearrange("b (j r) -> b j r", r=32)

            def emit_y_assembly(j_lo, j_hi, eng):
                """Assemble y blocks j in [j_lo, j_hi); requires V[t] for
                t <= j_hi (uses t=j+1 for the r=31 edge)."""
                jm = min(j_hi, 32)      # main1 defined for j<=31
                if jm > j_lo:
                    eng.tensor_add(
                        out=Y3[:, j_lo:jm, :], in0=V3[:, j_lo:jm, 1:33],
                        in1=B3[:, j_lo:jm, :])
                if j_hi == 33:          # last block: bias only here
                    eng.tensor_copy(
                        out=y_sb[:, 1024:1056], in_=brep_sb[:, 1024:1056])
                lo = max(1, j_lo)
                if j_hi > lo:           # += v[:, j-1, r+33]
                    eng.tensor_add(
                        out=Y3[:, lo:j_hi, :], in0=Y3[:, lo:j_hi, :],
                        in1=V3[:, lo - 1:j_hi - 1, 33:65])
                lo = max(2, j_lo)
                if j_hi > lo:           # r=0: += v[:, j-2, 65]
                    eng.tensor_add(
                        out=Y3[:, lo:j_hi, 0], in0=Y3[:, lo:j_hi, 0],
                        in1=V3[:, lo - 2:j_hi - 2, 65])
                hi = min(j_hi, 31)
                if hi > j_lo:           # r=31: += v[:, j+1, 0]
                    eng.tensor_add(
                        out=Y3[:, j_lo:hi, 31], in0=Y3[:, j_lo:hi, 31],
                        in1=V3[:, j_lo + 1:hi + 1, 0])

            # after vtrans(i) V[t] is final for t <= 4i+3, so y blocks
            # j < min(4i+3, 33) can assemble (block j reads up to t=j+1)
            y_wm = [0]

            def emit_rounds(i):
                if i < TC - 1:
                    j_hi = min(4 * i + 3, 33)
                    if j_hi > y_wm[0]:
                        emit_y_assembly(y_wm[0], j_hi, nc.gpsimd)
                        y_wm[0] = j_hi
                else:
                    # final round: split across gpsimd + vector (independent
                    # j ranges) to shorten the tail
                    mid = (y_wm[0] + 33 + 1) // 2
                    emit_y_assembly(y_wm[0], mid, nc.gpsimd)
                    emit_y_assembly(mid, 33, nc.vector)
                    y_wm[0] = 33
                # progressive stores once column ranges are final
                if y_wm[0] >= 15 and not store_done[0]:
                    nc.sync.dma_start(out=y[:, 0:480], in_=y_sb[:, 0:480])
                    store_done[0] = True
                if y_wm[0] >= 27 and not store_done[1]:
                    nc.sync.dma_start(out=y[:, 480:864], in_=y_sb[:, 480:864])
                    store_done[1] = True

            store_done = [False, False]

            prev = None          # (tcix, v_tiles) of the chunk awaiting fused
            emit_warmup()
            nc.sync.dma_start(out=b1c_sb[:], in_=b1c[:])
            emit_x_load(0, split_first=True)
            nc.sync.dma_start(out=w1t_sb[:], in_=w1t[:])
            emit_x_load_rest(0)
            emit_x_load(1)
            nc.sync.dma_start(out=g_sb[:], in_=g[:])
            nc.sync.dma_start(out=ident_sb[:], in_=ident[:])
            nc.sync.dma_start(out=brep_sb[:], in_=brep[:])
            for tcix in range(TC):
                ht_tiles[tcix] = {
                    (cp, hc): ht_pool.tile(
                        [128, 2 * CW], FH,
                        tag=f"ht{hc}_{cp}", name=f"ht_{tcix}_{hc}_{cp}")
                    for cp in range(C // 2) for hc in range(HC)}
                if tcix + 2 < TC:
                    emit_x_load(tcix + 2)
                # interleave: MLP1(tcix, cp) with fused(tcix-1, c) so PE
                # always has matmul work while relu copies drain PSUM
                for cp in range(C // 2):
                    emit_mlp1_pair(tcix, cp)
                    if prev is not None:
                        emit_fused(prev[0], prev[1], 2 * cp)
                        emit_fused(prev[0], prev[1], 2 * cp + 1)
                if prev is not None:
                    emit_vtrans(prev[0], prev[1])
                    emit_rounds(prev[0])
                del xin_tiles[tcix]
                v_tiles = [
                    (lo, hi, var, vps_pool.tile(
                        [66, CW], FD, tag="v_ps", name=f"v_ps_{tcix}_{lo}"))
                    for (lo, hi, var) in chunk_ranges(tcix)]
                prev = (tcix, v_tiles)
            for c in range(C):
                emit_fused(prev[0], prev[1], c)
            emit_vtrans(prev[0], prev[1])
            emit_rounds(prev[0])

            nc.sync.dma_start(out=y[:, 864:SIG_LEN], in_=y_sb[:, 864:SIG_LEN])

    nc.finalize()
    return nc


def make_in_maps(inputs):
    """Per-core input maps (shared by kernel(), sim checks, and bench)."""
    x = np.asarray(inputs["encoder_output"], dtype=np.float32)
    W1 = np.asarray(inputs["W1"], np.float32)
    b1 = np.asarray(inputs["b1"], np.float32)

    G, Bvec = _host_prep(
        inputs["W1"], inputs["b1"], inputs["W2"], inputs["b2"],
        inputs["Winv"], inputs["binv"], inputs["Wconv"], inputs["bconv"])

    # pack G -> [128, HC*C*3*66]: g_sb[p, _g_col(hc,c,var)+m] = G[var, c, hc*128+p, m]
    g_pack = np.zeros((128, HC * C * 3 * 66), np.float32)
    for hc in range(HC):
        for c in range(C):
            for var in range(3):
                col = _g_col(hc, c, var)
                g_pack[:, col:col + 66] = G[var, c, hc * 128:(hc + 1) * 128, :]

    w1t = np.ascontiguousarray(W1.T).astype(np.float16)     # [E, H]
    g_pack = g_pack.astype(np.float16)
    b1c = np.ascontiguousarray(b1.reshape(HC, 128).T)       # [128, HC]
    brep = np.ascontiguousarray(np.broadcast_to(Bvec, (BL, SIG_LEN)))
    ident = np.eye(128, dtype=np.float32)

    # [B,C,T,E] -> per-core [TC, E, (c, tl, b)] fp16 (device reads xT directly)
    xs = x.reshape(N_CORES, BL, C, TC, TL, E).transpose(0, 3, 5, 2, 4, 1)
    xs = np.ascontiguousarray(xs.astype(np.float16)).reshape(
        N_CORES, TC, E, C * CW)
    return [
        {
            "x": xs[i],
            "w1t": w1t, "b1c": b1c, "g": g_pack,
            "brep": brep, "ident": ident,
        }
        for i in range(N_CORES)
    ]


def kernel(**inputs) -> np.ndarray:
    if "nc" not in _CACHE:
        _CACHE["nc"] = _build_bass()
    nc = _CACHE["nc"]

    in_maps = make_in_maps(inputs)
    res = run_bass_kernel_spmd(nc, in_maps, core_ids=list(range(N_CORES)))
    _CACHE["last_result"] = res
    y = np.concatenate([r["y"] for r in res.results], axis=0)   # [B, 1056]
    return y.reshape(B, 1, SIG_LEN).astype(np.float32)


if __name__ == "__main__":
    rng = np.random.default_rng(0)
    ins = {
        "encoder_output": rng.standard_normal((B, C, T, E), dtype=np.float32),
        "W1": rng.standard_normal((H, E), dtype=np.float32) / np.sqrt(E),
        "b1": rng.standard_normal((H,), dtype=np.float32) / np.sqrt(E),
        "W2": rng.standard_normal((E, H), dtype=np.float32) / np.sqrt(H),
        "b2": rng.standard_normal((E,), dtype=np.float32) / np.sqrt(H),
        "Winv": rng.standard_normal((SEG_LEN, E), dtype=np.float32) / np.sqrt(E),
        "binv": rng.standard_normal((SEG_LEN,), dtype=np.float32) / np.sqrt(E),
        "Wconv": rng.standard_normal((1, C, 3), dtype=np.float32) / np.sqrt(C * 3),
        "bconv": rng.standard_normal((1,), dtype=np.float32) / np.sqrt(C * 3),
    }
    out = kernel(**ins)
    print("kernel output", out.shape, out.dtype)


# revision 22
# speedup vs baseline: 1.6073x; 1.0590x over previous
"""Trainium2 Bass kernel for nn_Decoder (MLP -> inverse token embedding ->
overlap-add -> channel-merge conv), data-parallel over batch on 8 NeuronCores.

Self-contained: hardcodes shapes; host-side numpy folds everything after the
first Linear+ReLU into per-channel fused matrices G (W2 -> Winv -> overlap-add
normalization -> 3-tap channel conv), so the device pipeline is:

    xT[E,tok] (host pre-transposed, fp16) --matmul W1T--> h[Hc,tok] in PSUM
    --ACT/DVE relu+bias--> hT in SBUF --matmul G (accum over c,Hc)--> v[66,tok]
    --PE transpose--> vT[b,66] --strided adds (overlap-add)--> y[b,1056]

Sharding: batch 1024 -> 8 cores x 128.
"""

import numpy as np

import concourse.bacc as bacc
import concourse.mybir as mybir
from concourse.bass_utils import run_bass_kernel_spmd
from concourse.tile import TileContext

# problem shapes (hardcoded per contract)
B, C, T, E, H = 1024, 8, 32, 128, 256
SEG_LEN, SIG_LEN, NUM_SEG, STEP = 64, 1056, 32, 32
N_CORES = 8
BL = B // N_CORES          # local batch per core = 128
HC = H // 128              # H chunks = 2
TC = 8                     # t-chunks
TL = T // TC               # t per chunk = 4
CW = TL * 128              # columns per (c, t-chunk) = 512
FD = mybir.dt.float32
FR = mybir.dt.float32r   # fp32 storage, FP22 multiply: 4x faster PE
FH = mybir.dt.float16

_CACHE = {}


def _host_prep(W1, b1, W2, b2, Winv, binv, Wconv, bconv):
    """Fold W2/Winv/normalization/conv into G [3var][C][H,66] and bias B[1056]."""
    counter = np.zeros(SIG_LEN, np.float64)
    for t in range(NUM_SEG):
        counter[t * STEP: t * STEP + SEG_LEN] += 1.0
    n = 1.0 / counter

    F = Winv.astype(np.float64) @ W2.astype(np.float64)          # [64, H]
    binv2 = Winv.astype(np.float64) @ b2.astype(np.float64) + binv.astype(np.float64)
    Wc = Wconv[0].astype(np.float64)                             # [C, 3]

    def n_of(var, s):
        if var == 0:
            return n[s]
        if var == 2:
            return n[992 + s]
        return 0.5

    G = np.zeros((3, C, H, 66), np.float64)
    for var in range(3):
        for c in range(C):
            for m_idx in range(66):
                for k in range(3):
                    s = m_idx + k - 2
                    if 0 <= s < SEG_LEN:
                        G[var, c, :, m_idx] += Wc[c, k] * n_of(var, s) * F[s, :]

    sig_b = np.zeros(SIG_LEN, np.float64)
    for t in range(NUM_SEG):
        sig_b[t * STEP: t * STEP + SEG_LEN] += binv2
    sig_b *= n
    Bvec = np.full(SIG_LEN, float(np.asarray(bconv).reshape(-1)[0]), np.float64)
    q = np.arange(SIG_LEN)
    for k in range(3):
        qq = q + k - 1
        valid = (qq >= 0) & (qq < SIG_LEN)
        for c in range(C):
            Bvec[valid] += Wc[c, k] * sig_b[qq[valid]]
    return G.astype(np.float32), Bvec.astype(np.float32)


def _g_col(hc, c, var):
    """Column offset of G slice (hc, c, var) inside g_sb [128, 2*8*3*66]."""
    return ((hc * C + c) * 3 + var) * 66


def _build_bass():
    nc = bacc.Bacc("TRN2")

    # host pre-transposed to [TC, E, C*TL*BL] fp16: per t-chunk one contiguous
    # [128, 4096] block, columns ordered (c, tl, b)
    x = nc.dram_tensor("x", [TC, E, C * CW], FH, kind="ExternalInput")
    w1t = nc.dram_tensor("w1t", [E, H], FH, kind="ExternalInput")
    b1c = nc.dram_tensor("b1c", [128, HC], FD, kind="ExternalInput")
    g = nc.dram_tensor("g", [128, HC * C * 3 * 66], FH, kind="ExternalInput")
    brep = nc.dram_tensor("brep", [BL, SIG_LEN], FD, kind="ExternalInput")
    ident = nc.dram_tensor("ident", [128, 128], FR, kind="ExternalInput")
    y = nc.dram_tensor("y", [BL, SIG_LEN], FD, kind="ExternalOutput")

    with TileContext(nc) as tc:
        with (
            tc.tile_pool(name="consts", bufs=1) as consts,
            tc.tile_pool(name="xin", bufs=3) as xin_pool,
            tc.tile_pool(name="ht", bufs=2) as ht_pool,
            tc.tile_pool(name="vsb", bufs=3) as vsb_pool,
            tc.tile_pool(name="big", bufs=1) as big_pool,
            tc.tile_pool(name="pe_out", bufs=1, space="PSUM") as peout_pool,
            tc.tile_pool(name="h_ps", bufs=4, space="PSUM") as hps_pool,
            tc.tile_pool(name="v_ps", bufs=3, space="PSUM") as vps_pool,
        ):
            w1t_sb = consts.tile([E, H], FH)
            b1c_sb = consts.tile([128, HC], FD)
            g_sb = consts.tile([128, HC * C * 3 * 66], FH)
            ident_sb = consts.tile([128, 128], FR)
            warm_a = consts.tile([128, 128], FH)
            warm_b = consts.tile([128, 384], FH)
            brep_sb = big_pool.tile([BL, SIG_LEN], FD)

            V_sb = big_pool.tile([BL, T * 66], FD)      # v transposed: [b, t*66+m]
            y_sb = big_pool.tile([BL, SIG_LEN], FD)

            xin_tiles = {}

            def emit_x_load(tcix, split_first=False):
                xt = xin_pool.tile([E, C * CW], FH, tag="xin")
                if split_first:
                    # first chunk: land c0-c1 early so MLP1 starts sooner
                    nc.sync.dma_start(out=xt[:, 0:2 * CW],
                                      in_=x[tcix, :, 0:2 * CW])
                    xin_tiles[tcix] = (xt, True)
                    return
                nc.sync.dma_start(out=xt[:], in_=x[tcix, :, :])
                xin_tiles[tcix] = (xt, False)

            def emit_x_load_rest(tcix):
                xt, _ = xin_tiles[tcix]
                nc.sync.dma_start(out=xt[:, 2 * CW:C * CW],
                                  in_=x[tcix, :, 2 * CW:C * CW])

            # PE warm-up: the HAM clock gate releases only after ~3.4us of
            # sustained PE activity; burn garbage matmuls during the DMA head
            # so real matmuls start at 2.4 GHz. Reuses an h_ps buffer (WAW on
            # the in-order PE queue, so no stall).
            def emit_warmup():
                warm_ps = hps_pool.tile([128, CW], FD, tag="h_ps",
                                        name="warm_ps")
                nc.gpsimd.memset(warm_a[:], 1.0)
                nc.gpsimd.memset(warm_b[:], 1.0)
                # trigger ACT_TABLE_LOAD now, not at the first real relu
                nc.scalar.activation(
                    warm_a[:], warm_a[:],
                    mybir.ActivationFunctionType.Relu, scale=1.0)
                for _ in range(14):
                    nc.tensor.matmul(
                        warm_ps[:, 0:384], warm_b[:, 0:128], warm_b[:],
                        start=True, stop=True)

            # software pipeline: fused stage runs one t-chunk behind MLP1
            ht_tiles = {}

            # greedy ACT/DVE load balancer for PSUM->SBUF copies and relus
            eng_busy = {"act": 0.0, "dve": 0.0}

            def pick_engine(fd):
                ca = (352 + fd) / 1.2
                cd = (151 + fd) / 0.96
                if eng_busy["act"] + ca <= eng_busy["dve"] + cd:
                    eng_busy["act"] += ca
                    return "act"
                eng_busy["dve"] += cd
                return "dve"

            def bal_copy(out, in_, fd):
                if pick_engine(fd) == "act":
                    nc.scalar.copy(out=out, in_=in_)
                else:
                    nc.vector.tensor_copy(out=out, in_=in_)

            def chunk_ranges(tcix):
                # column ranges with uniform G variant; cols = tl*128 + b
                if tcix == 0:
                    return [(0, 128, 0), (128, CW, 1)]        # t=0 -> var 0
                if tcix == TC - 1:
                    return [(0, 384, 1), (384, CW, 2)]        # t=31 -> var 2
                return [(0, CW, 1)]

            def emit_mlp1(tcix, c):
                """MLP1 for one channel: 2 matmuls (one per hc) into 1-bank
                PSUM tiles; bufs=4 gives two channels of pipeline depth so
                the drains never stall the PE."""
                ht = ht_tiles[tcix]
                xt, _ = xin_tiles[tcix]
                cp, half = divmod(c, 2)
                h_list = []
                for hc in range(HC):
                    h_ps = hps_pool.tile([128, CW], FD, tag="h_ps",
                                         name=f"h_ps_{tcix}_{c}_{hc}")
                    nc.tensor.matmul(
                        h_ps[:],
                        w1t_sb[:, hc * 128:(hc + 1) * 128],
                        xt[:, c * CW:(c + 1) * CW],
                        start=True, stop=True,
                    )
                    h_list.append(h_ps)
                for hc in range(HC):
                    dst = ht[(cp, hc)][:, half * CW:(half + 1) * CW]
                    src = h_list[hc][:]
                    if pick_engine(CW) == "act":
                        nc.scalar.activation(
                            dst, src,
                            mybir.ActivationFunctionType.Relu,
                            bias=b1c_sb[:, hc:hc + 1], scale=1.0,
                        )
                    else:
                        nc.vector.tensor_scalar(
                            dst, src,
                            b1c_sb[:, hc:hc + 1], 0.0,
                            mybir.AluOpType.add, mybir.AluOpType.max,
                        )

            def emit_fused(tcix, v_tiles, c):
                """fused G matmuls for channel c accumulating into v_tiles."""
                ht = ht_tiles[tcix]
                cp, half = divmod(c, 2)
                for (lo, hi, var, v_ps) in v_tiles:
                    for hc in range(HC):
                        i = c * HC + hc
                        nc.tensor.matmul(
                            v_ps[:, lo:hi],
                            g_sb[:, _g_col(hc, c, var):_g_col(hc, c, var) + 66],
                            ht[(cp, hc)][:, half * CW + lo:half * CW + hi],
                            start=(i == 0), stop=(i == C * HC - 1),
                        )

            def emit_vtrans(tcix, v_tiles):
                """copy v psum -> sbuf, PE-transpose per t into one PSUM tile,
                single merged copy into V_sb."""
                del ht_tiles[tcix]
                v_sb = vsb_pool.tile([66, CW], FR, tag="v_sb")
                for (lo, hi, var, v_ps) in v_tiles:
                    bal_copy(v_sb[:, lo:hi], v_ps[:, lo:hi], hi - lo)
                vt_ps = peout_pool.tile([128, 264], FR, tag="pe_out")
                for tl in range(TL):
                    nc.tensor.transpose(
                        vt_ps[:, tl * 66:(tl + 1) * 66],
                        v_sb[:, tl * 128:(tl + 1) * 128],
                        ident_sb[0:66, 0:66],
                    )
                bal_copy(V_sb[:, tcix * 264:(tcix + 1) * 264], vt_ps[:], 264)

            # overlap-add assembly in rounds (per watermark) so it overlaps
            # with later chunks instead of serializing at the end
            V3 = V_sb[:].rearrange("b (t m) -> b t m", m=66)
            Y3 = y_sb[:].rearrange("b (j r) -> b j r", r=32)
            B3 = brep_sb[:].rearrange("b (j r) -> b j r", r=32)

            def emit_y_assembly(j_lo, j_hi, eng):
                """Assemble y blocks j in [j_lo, j_hi); requires V[t] for
                t <= j_hi (uses t=j+1 for the r=31 edge)."""
                jm = min(j_hi, 32)      # main1 defined for j<=31
                if jm > j_lo:
                    eng.tensor_add(
                        out=Y3[:, j_lo:jm, :], in0=V3[:, j_lo:jm, 1:33],
                        in1=B3[:, j_lo:jm, :])
                if j_hi == 33:          # last block: bias only here
                    eng.tensor_copy(
                        out=y_sb[:, 1024:1056], in_=brep_sb[:, 1024:1056])
                lo = max(1, j_lo)
                if j_hi > lo:           # += v[:, j-1, r+33]
                    eng.tensor_add(
                        out=Y3[:, lo:j_hi, :], in0=Y3[:, lo:j_hi, :],
                        in1=V3[:, lo - 1:j_hi - 1, 33:65])
                lo = max(2, j_lo)
                if j_hi > lo:           # r=0: += v[:, j-2, 65]
                    eng.tensor_add(
                        out=Y3[:, lo:j_hi, 0], in0=Y3[:, lo:j_hi, 0],
                        in1=V3[:, lo - 2:j_hi - 2, 65])
                hi = min(j_hi, 31)
                if hi > j_lo:           # r=31: += v[:, j+1, 0]
                    eng.tensor_add(
                        out=Y3[:, j_lo:hi, 31], in0=Y3[:, j_lo:hi, 31],
                        in1=V3[:, j_lo + 1:hi + 1, 0])

            # after vtrans(i) V[t] is final for t <= 4i+3, so y blocks
            # j < min(4i+3, 33) can assemble (block j reads up to t=j+1)
            y_wm = [0]

            def emit_rounds(i):
                if i < TC - 1:
                    j_hi = min(4 * i + 3, 33)
                    if j_hi > y_wm[0]:
                        emit_y_assembly(y_wm[0], j_hi, nc.gpsimd)
                        y_wm[0] = j_hi
                else:
                    # final round: split across gpsimd + vector (independent
                    # j ranges) to shorten the tail
                    mid = (y_wm[0] + 33 + 1) // 2
                    emit_y_assembly(y_wm[0], mid, nc.gpsimd)
                    emit_y_assembly(mid, 33, nc.vector)
                    y_wm[0] = 33
                # progressive stores once column ranges are final
                if y_wm[0] >= 15 and not store_done[0]:
                    nc.sync.dma_start(out=y[:, 0:480], in_=y_sb[:, 0:480])
                    store_done[0] = True
                if y_wm[0] >= 27 and not store_done[1]:
                    nc.sync.dma_start(out=y[:, 480:864], in_=y_sb[:, 480:864])
                    store_done[1] = True

            store_done = [False, False]

            prev = None          # (tcix, v_tiles) of the chunk awaiting fused
            emit_warmup()
            nc.sync.dma_start(out=b1c_sb[:], in_=b1c[:])
            emit_x_load(0, split_first=True)
            nc.sync.dma_start(out=w1t_sb[:], in_=w1t[:])
            emit_x_load_rest(0)
            emit_x_load(1)
            nc.sync.dma_start(out=g_sb[:], in_=g[:])
            nc.sync.dma_start(out=ident_sb[:], in_=ident[:])
            nc.sync.dma_start(out=brep_sb[:], in_=brep[:])
            for tcix in range(TC):
                ht_tiles[tcix] = {
                    (cp, hc): ht_pool.tile(
                        [128, 2 * CW], FH,
                        tag=f"ht{hc}_{cp}", name=f"ht_{tcix}_{hc}_{cp}")
                    for cp in range(C // 2) for hc in range(HC)}
                if tcix + 2 < TC:
                    emit_x_load(tcix + 2)
                # interleave: MLP1(tcix, cp) with fused(tcix-1, c) so PE
                # always has matmul work while relu copies drain PSUM
                for c in range(C):
                    emit_mlp1(tcix, c)
                    if prev is not None:
                        emit_fused(prev[0], prev[1], c)
                if prev is not None:
                    emit_vtrans(prev[0], prev[1])
                    emit_rounds(prev[0])
                del xin_tiles[tcix]
                v_tiles = [
                    (lo, hi, var, vps_pool.tile(
                        [66, CW], FD, tag="v_ps", name=f"v_ps_{tcix}_{lo}"))
                    for (lo, hi, var) in chunk_ranges(tcix)]
                prev = (tcix, v_tiles)
            for c in range(C):
                emit_fused(prev[0], prev[1], c)
            emit_vtrans(prev[0], prev[1])
            emit_rounds(prev[0])

            nc.sync.dma_start(out=y[:, 864:SIG_LEN], in_=y_sb[:, 864:SIG_LEN])

    nc.finalize()
    return nc


def make_in_maps(inputs):
    """Per-core input maps (shared by kernel(), sim checks, and bench)."""
    x = np.asarray(inputs["encoder_output"], dtype=np.float32)
    W1 = np.asarray(inputs["W1"], np.float32)
    b1 = np.asarray(inputs["b1"], np.float32)

    G, Bvec = _host_prep(
        inputs["W1"], inputs["b1"], inputs["W2"], inputs["b2"],
        inputs["Winv"], inputs["binv"], inputs["Wconv"], inputs["bconv"])

    # pack G -> [128, HC*C*3*66]: g_sb[p, _g_col(hc,c,var)+m] = G[var, c, hc*128+p, m]
    g_pack = np.zeros((128, HC * C * 3 * 66), np.float32)
    for hc in range(HC):
        for c in range(C):
            for var in range(3):
                col = _g_col(hc, c, var)
                g_pack[:, col:col + 66] = G[var, c, hc * 128:(hc + 1) * 128, :]

    w1t = np.ascontiguousarray(W1.T).astype(np.float16)     # [E, H]
    g_pack = g_pack.astype(np.float16)
    b1c = np.ascontiguousarray(b1.reshape(HC, 128).T)       # [128, HC]
    brep = np.ascontiguousarray(np.broadcast_to(Bvec, (BL, SIG_LEN)))
    ident = np.eye(128, dtype=np.float32)

    # [B,C,T,E] -> per-core [TC, E, (c, tl, b)] fp16 (device reads xT directly)
    xs = x.reshape(N_CORES, BL, C, TC, TL, E).transpose(0, 3, 5, 2, 4, 1)
    xs = np.ascontiguousarray(xs.astype(np.float16)).reshape(
        N_CORES, TC, E, C * CW)
    return [
        {
            "x": xs[i],
            "w1t": w1t, "b1c": b1c, "g": g_pack,
            "brep": brep, "ident": ident,
        }
        for i in range(N_CORES)
    ]


def kernel(**inputs) -> np.ndarray:
    if "nc" not in _CACHE:
        _CACHE["nc"] = _build_bass()
    nc = _CACHE["nc"]

    in_maps = make_in_maps(inputs)
    res = run_bass_kernel_spmd(nc, in_maps, core_ids=list(range(N_CORES)))
    _CACHE["last_result"] = res
    y = np.concatenate([r["y"] for r in res.results], axis=0)   # [B, 1056]
    return y.reshape(B, 1, SIG_LEN).astype(np.float32)


if __name__ == "__main__":
    rng = np.random.default_rng(0)
    ins = {
        "encoder_output": rng.standard_normal((B, C, T, E), dtype=np.float32),
        "W1": rng.standard_normal((H, E), dtype=np.float32) / np.sqrt(E),
        "b1": rng.standard_normal((H,), dtype=np.float32) / np.sqrt(E),
        "W2": rng.standard_normal((E, H), dtype=np.float32) / np.sqrt(H),
        "b2": rng.standard_normal((E,), dtype=np.float32) / np.sqrt(H),
        "Winv": rng.standard_normal((SEG_LEN, E), dtype=np.float32) / np.sqrt(E),
        "binv": rng.standard_normal((SEG_LEN,), dtype=np.float32) / np.sqrt(E),
        "Wconv": rng.standard_normal((1, C, 3), dtype=np.float32) / np.sqrt(C * 3),
        "bconv": rng.standard_normal((1,), dtype=np.float32) / np.sqrt(C * 3),
    }
    out = kernel(**ins)
    print("kernel output", out.shape, out.dtype)


# revision 27
# speedup vs baseline: 1.6113x; 1.0025x over previous
"""Trainium2 Bass kernel for nn_Decoder (MLP -> inverse token embedding ->
overlap-add -> channel-merge conv), data-parallel over batch on 8 NeuronCores.

Self-contained: hardcodes shapes; host-side numpy folds everything after the
first Linear+ReLU into per-channel fused matrices G (W2 -> Winv -> overlap-add
normalization -> 3-tap channel conv), so the device pipeline is:

    xT[E,tok] (host pre-transposed, fp16) --matmul W1T--> h[Hc,tok] in PSUM
    --ACT/DVE relu+bias--> hT in SBUF --matmul G (accum over c,Hc)--> v[66,tok]
    --PE transpose--> vT[b,66] --strided adds (overlap-add)--> y[b,1056]

Sharding: batch 1024 -> 8 cores x 128.
"""

import numpy as np

import concourse.bacc as bacc
import concourse.mybir as mybir
from concourse.bass_utils import run_bass_kernel_spmd
from concourse.tile import TileContext

# problem shapes (hardcoded per contract)
B, C, T, E, H = 1024, 8, 32, 128, 256
SEG_LEN, SIG_LEN, NUM_SEG, STEP = 64, 1056, 32, 32
N_CORES = 8
BL = B // N_CORES          # local batch per core = 128
HC = H // 128              # H chunks = 2
TC = 8                     # t-chunks
TL = T // TC               # t per chunk = 4
CW = TL * 128              # columns per (c, t-chunk) = 512
FD = mybir.dt.float32
FR = mybir.dt.float32r   # fp32 storage, FP22 multiply: 4x faster PE
FH = mybir.dt.float16

_CACHE = {}


def _host_prep(W1, b1, W2, b2, Winv, binv, Wconv, bconv):
    """Fold W2/Winv/normalization/conv into G [3var][C][H,66] and bias B[1056]."""
    counter = np.zeros(SIG_LEN, np.float64)
    for t in range(NUM_SEG):
        counter[t * STEP: t * STEP + SEG_LEN] += 1.0
    n = 1.0 / counter

    F = Winv.astype(np.float64) @ W2.astype(np.float64)          # [64, H]
    binv2 = Winv.astype(np.float64) @ b2.astype(np.float64) + binv.astype(np.float64)
    Wc = Wconv[0].astype(np.float64)                             # [C, 3]

    def n_of(var, s):
        if var == 0:
            return n[s]
        if var == 2:
            return n[992 + s]
        return 0.5

    G = np.zeros((3, C, H, 66), np.float64)
    for var in range(3):
        for c in range(C):
            for m_idx in range(66):
                for k in range(3):
                    s = m_idx + k - 2
                    if 0 <= s < SEG_LEN:
                        G[var, c, :, m_idx] += Wc[c, k] * n_of(var, s) * F[s, :]

    sig_b = np.zeros(SIG_LEN, np.float64)
    for t in range(NUM_SEG):
        sig_b[t * STEP: t * STEP + SEG_LEN] += binv2
    sig_b *= n
    Bvec = np.full(SIG_LEN, float(np.asarray(bconv).reshape(-1)[0]), np.float64)
    q = np.arange(SIG_LEN)
    for k in range(3):
        qq = q + k - 1
        valid = (qq >= 0) & (qq < SIG_LEN)
        for c in range(C):
            Bvec[valid] += Wc[c, k] * sig_b[qq[valid]]
    return G.astype(np.float32), Bvec.astype(np.float32)


def _g_col(hc, c, var):
    """Column offset of G slice (hc, c, var) inside g_sb [128, 2*8*3*66]."""
    return ((hc * C + c) * 3 + var) * 66


def _build_bass():
    nc = bacc.Bacc("TRN2")

    # host pre-transposed to [TC, E, C*TL*BL] fp16: per t-chunk one contiguous
    # [128, 4096] block, columns ordered (c, tl, b)
    x = nc.dram_tensor("x", [TC, E, C * CW], FH, kind="ExternalInput")
    w1t = nc.dram_tensor("w1t", [E, H], FH, kind="ExternalInput")
    b1c = nc.dram_tensor("b1c", [128, HC], FD, kind="ExternalInput")
    g = nc.dram_tensor("g", [128, HC * C * 3 * 66], FH, kind="ExternalInput")
    brep = nc.dram_tensor("brep", [BL, SIG_LEN], FD, kind="ExternalInput")
    ident = nc.dram_tensor("ident", [128, 128], FR, kind="ExternalInput")
    y = nc.dram_tensor("y", [BL, SIG_LEN], FD, kind="ExternalOutput")
    scr = nc.dram_tensor("scr", [1, 4], FD, kind="Internal")

    with TileContext(nc) as tc:
        with (
            tc.tile_pool(name="consts", bufs=1) as consts,
            tc.tile_pool(name="xin", bufs=3) as xin_pool,
            tc.tile_pool(name="ht", bufs=2) as ht_pool,
            tc.tile_pool(name="vsb", bufs=3) as vsb_pool,
            tc.tile_pool(name="big", bufs=1) as big_pool,
            tc.tile_pool(name="pe_out", bufs=1, space="PSUM") as peout_pool,
            tc.tile_pool(name="h_ps", bufs=4, space="PSUM") as hps_pool,
            tc.tile_pool(name="v_ps", bufs=3, space="PSUM") as vps_pool,
        ):
            w1t_sb = consts.tile([E, H], FH)
            b1c_sb = consts.tile([128, HC], FD)
            g_sb = consts.tile([128, HC * C * 3 * 66], FH)
            ident_sb = consts.tile([128, 128], FR)
            warm_a = consts.tile([128, 128], FH)
            warm_b = consts.tile([128, 384], FH)
            brep_sb = big_pool.tile([BL, SIG_LEN], FD)

            V_sb = big_pool.tile([BL, T * 66], FD)      # v transposed: [b, t*66+m]
            y_sb = big_pool.tile([BL, SIG_LEN], FD)

            xin_tiles = {}

            def emit_x_load(tcix, split_first=False):
                xt = xin_pool.tile([E, C * CW], FH, tag="xin")
                if split_first:
                    # first chunk in 4 pieces: more DMA-queue share against
                    # the const loads, and c0-c1 land early so MLP1 starts
                    nc.sync.dma_start(out=xt[:, 0:2 * CW],
                                      in_=x[tcix, :, 0:2 * CW])
                    xin_tiles[tcix] = (xt, True)
                    return
                nc.sync.dma_start(out=xt[:], in_=x[tcix, :, :])
                xin_tiles[tcix] = (xt, False)

            def emit_x_load_rest(tcix):
                xt, _ = xin_tiles[tcix]
                for k in range(1, 4):
                    nc.sync.dma_start(
                        out=xt[:, 2 * k * CW:2 * (k + 1) * CW],
                        in_=x[tcix, :, 2 * k * CW:2 * (k + 1) * CW])

            # PE warm-up: the HAM clock gate releases only after ~3.4us of
            # sustained PE activity; burn garbage matmuls during the DMA head
            # so real matmuls start at 2.4 GHz. Reuses an h_ps buffer (WAW on
            # the in-order PE queue, so no stall).
            def emit_warmup():
                warm_ps = hps_pool.tile([128, CW], FD, tag="h_ps",
                                        name="warm_ps")
                nc.gpsimd.memset(warm_a[:], 1.0)
                nc.gpsimd.memset(warm_b[:], 1.0)
                # trigger ACT_TABLE_LOAD now, not at the first real relu
                nc.scalar.activation(
                    warm_a[:], warm_a[:],
                    mybir.ActivationFunctionType.Relu, scale=1.0)
                for _ in range(14):
                    nc.tensor.matmul(
                        warm_ps[:, 0:384], warm_b[:, 0:128], warm_b[:],
                        start=True, stop=True)

            # software pipeline: fused stage runs one t-chunk behind MLP1
            ht_tiles = {}

            # greedy ACT/DVE load balancer for PSUM->SBUF copies and relus
            eng_busy = {"act": 0.0, "dve": 0.0}

            def pick_engine(fd):
                ca = (352 + fd) / 1.2
                cd = (151 + fd) / 0.96
                if eng_busy["act"] + ca <= eng_busy["dve"] + cd:
                    eng_busy["act"] += ca
                    return "act"
                eng_busy["dve"] += cd
                return "dve"

            def bal_copy(out, in_, fd):
                if pick_engine(fd) == "act":
                    nc.scalar.copy(out=out, in_=in_)
                else:
                    nc.vector.tensor_copy(out=out, in_=in_)

            def chunk_ranges(tcix):
                # column ranges with uniform G variant; cols = tl*128 + b
                if tcix == 0:
                    return [(0, 128, 0), (128, CW, 1)]        # t=0 -> var 0
                if tcix == TC - 1:
                    return [(0, 384, 1), (384, CW, 2)]        # t=31 -> var 2
                return [(0, CW, 1)]

            def emit_mlp1(tcix, c):
                """MLP1 for one channel: 2 matmuls (one per hc) into 1-bank
                PSUM tiles; bufs=4 gives two channels of pipeline depth so
                the drains never stall the PE."""
                ht = ht_tiles[tcix]
                xt, _ = xin_tiles[tcix]
                cp, half = divmod(c, 2)
                h_list = []
                for hc in range(HC):
                    h_ps = hps_pool.tile([128, CW], FD, tag="h_ps",
                                         name=f"h_ps_{tcix}_{c}_{hc}")
                    nc.tensor.matmul(
                        h_ps[:],
                        w1t_sb[:, hc * 128:(hc + 1) * 128],
                        xt[:, c * CW:(c + 1) * CW],
                        start=True, stop=True,
                    )
                    h_list.append(h_ps)
                for hc in range(HC):
                    dst = ht[(cp, hc)][:, half * CW:(half + 1) * CW]
                    src = h_list[hc][:]
                    if pick_engine(CW) == "act":
                        nc.scalar.activation(
                            dst, src,
                            mybir.ActivationFunctionType.Relu,
                            bias=b1c_sb[:, hc:hc + 1], scale=1.0,
                        )
                    else:
                        nc.vector.tensor_scalar(
                            dst, src,
                            b1c_sb[:, hc:hc + 1], 0.0,
                            mybir.AluOpType.add, mybir.AluOpType.max,
                        )

            def emit_fused(tcix, v_tiles, c):
                """fused G matmuls for channel c accumulating into v_tiles."""
                ht = ht_tiles[tcix]
                cp, half = divmod(c, 2)
                for (lo, hi, var, v_ps) in v_tiles:
                    for hc in range(HC):
                        i = c * HC + hc
                        nc.tensor.matmul(
                            v_ps[:, lo:hi],
                            g_sb[:, _g_col(hc, c, var):_g_col(hc, c, var) + 66],
                            ht[(cp, hc)][:, half * CW + lo:half * CW + hi],
                            start=(i == 0), stop=(i == C * HC - 1),
                        )

            def emit_vtrans(tcix, v_tiles):
                """copy v psum -> sbuf, PE-transpose per t into one PSUM tile,
                single merged copy into V_sb."""
                del ht_tiles[tcix]
                v_sb = vsb_pool.tile([66, CW], FR, tag="v_sb")
                for (lo, hi, var, v_ps) in v_tiles:
                    bal_copy(v_sb[:, lo:hi], v_ps[:, lo:hi], hi - lo)
                vt_ps = peout_pool.tile([128, 264], FR, tag="pe_out")
                for tl in range(TL):
                    nc.tensor.transpose(
                        vt_ps[:, tl * 66:(tl + 1) * 66],
                        v_sb[:, tl * 128:(tl + 1) * 128],
                        ident_sb[0:66, 0:66],
                    )
                bal_copy(V_sb[:, tcix * 264:(tcix + 1) * 264], vt_ps[:], 264)

            # overlap-add assembly in rounds (per watermark) so it overlaps
            # with later chunks instead of serializing at the end
            V3 = V_sb[:].rearrange("b (t m) -> b t m", m=66)
            Y3 = y_sb[:].rearrange("b (j r) -> b j r", r=32)
            B3 = brep_sb[:].rearrange("b (j r) -> b j r", r=32)

            def emit_y_assembly(j_lo, j_hi, eng):
                """Assemble y blocks j in [j_lo, j_hi); requires V[t] for
                t <= j_hi (uses t=j+1 for the r=31 edge)."""
                jm = min(j_hi, 32)      # main1 defined for j<=31
                if jm > j_lo:
                    eng.tensor_add(
                        out=Y3[:, j_lo:jm, :], in0=V3[:, j_lo:jm, 1:33],
                        in1=B3[:, j_lo:jm, :])
                if j_hi == 33:          # last block: bias only here
                    eng.tensor_copy(
                        out=y_sb[:, 1024:1056], in_=brep_sb[:, 1024:1056])
                lo = max(1, j_lo)
                if j_hi > lo:           # += v[:, j-1, r+33]
                    eng.tensor_add(
                        out=Y3[:, lo:j_hi, :], in0=Y3[:, lo:j_hi, :],
                        in1=V3[:, lo - 1:j_hi - 1, 33:65])
                lo = max(2, j_lo)
                if j_hi > lo:           # r=0: += v[:, j-2, 65]
                    eng.tensor_add(
                        out=Y3[:, lo:j_hi, 0], in0=Y3[:, lo:j_hi, 0],
                        in1=V3[:, lo - 2:j_hi - 2, 65])
                hi = min(j_hi, 31)
                if hi > j_lo:           # r=31: += v[:, j+1, 0]
                    eng.tensor_add(
                        out=Y3[:, j_lo:hi, 31], in0=Y3[:, j_lo:hi, 31],
                        in1=V3[:, j_lo + 1:hi + 1, 0])

            # after vtrans(i) V[t] is final for t <= 4i+3, so y blocks
            # j < min(4i+3, 33) can assemble (block j reads up to t=j+1)
            y_wm = [0]

            def emit_rounds(i):
                if i < TC - 1:
                    j_hi = min(4 * i + 3, 33)
                    if j_hi > y_wm[0]:
                        emit_y_assembly(y_wm[0], j_hi, nc.gpsimd)
                        y_wm[0] = j_hi
                else:
                    # final round: split across gpsimd + vector (independent
                    # j ranges) to shorten the tail
                    mid = (y_wm[0] + 33 + 1) // 2
                    emit_y_assembly(y_wm[0], mid, nc.gpsimd)
                    emit_y_assembly(mid, 33, nc.vector)
                    y_wm[0] = 33
                # progressive stores once column ranges are final (on the
                # gpsimd queue so their waits never block Sync x-dispatches)
                if y_wm[0] >= 15 and not store_done[0]:
                    nc.gpsimd.dma_start(out=y[:, 0:480], in_=y_sb[:, 0:480])
                    store_done[0] = True
                if y_wm[0] >= 27 and not store_done[1]:
                    nc.gpsimd.dma_start(out=y[:, 480:864], in_=y_sb[:, 480:864])
                    store_done[1] = True

            store_done = [False, False]

            prev = None          # (tcix, v_tiles) of the chunk awaiting fused
            emit_warmup()
            nc.sync.dma_start(out=b1c_sb[:], in_=b1c[:])
            emit_x_load(0, split_first=True)
            nc.sync.dma_start(out=w1t_sb[:], in_=w1t[:])
            emit_x_load_rest(0)
            emit_x_load(1)
            nc.sync.dma_start(out=g_sb[:], in_=g[:])
            nc.sync.dma_start(out=ident_sb[:], in_=ident[:])
            nc.sync.dma_start(out=brep_sb[:], in_=brep[:])
            # fence: block later x-chunk dispatches (emitted inside the loop)
            # until the head-critical loads above have drained, so they don't
            # steal SDMA round-robin bandwidth from x0/x1/g
            nc.sync.dma_start(out=scr[:], in_=brep_sb[0:1, 0:4])
            for tcix in range(TC):
                ht_tiles[tcix] = {
                    (cp, hc): ht_pool.tile(
                        [128, 2 * CW], FH,
                        tag=f"ht{hc}_{cp}", name=f"ht_{tcix}_{hc}_{cp}")
                    for cp in range(C // 2) for hc in range(HC)}
                if tcix + 2 < TC:
                    emit_x_load(tcix + 2)
                # interleave: MLP1(tcix, cp) with fused(tcix-1, c) so PE
                # always has matmul work while relu copies drain PSUM
                for c in range(C):
                    emit_mlp1(tcix, c)
                    if prev is not None:
                        emit_fused(prev[0], prev[1], c)
                if prev is not None:
                    emit_vtrans(prev[0], prev[1])
                    emit_rounds(prev[0])
                del xin_tiles[tcix]
                v_tiles = [
                    (lo, hi, var, vps_pool.tile(
                        [66, CW], FD, tag="v_ps", name=f"v_ps_{tcix}_{lo}"))
                    for (lo, hi, var) in chunk_ranges(tcix)]
                prev = (tcix, v_tiles)
            for c in range(C):
                emit_fused(prev[0], prev[1], c)
            emit_vtrans(prev[0], prev[1])
            emit_rounds(prev[0])

            nc.gpsimd.dma_start(out=y[:, 864:SIG_LEN], in_=y_sb[:, 864:SIG_LEN])

    nc.finalize()
    return nc


def make_in_maps(inputs):
    """Per-core input maps (shared by kernel(), sim checks, and bench)."""
    x = np.asarray(inputs["encoder_output"], dtype=np.float32)
    W1 = np.asarray(inputs["W1"], np.float32)
    b1 = np.asarray(inputs["b1"], np.float32)

    G, Bvec = _host_prep(
        inputs["W1"], inputs["b1"], inputs["W2"], inputs["b2"],
        inputs["Winv"], inputs["binv"], inputs["Wconv"], inputs["bconv"])

    # pack G -> [128, HC*C*3*66]: g_sb[p, _g_col(hc,c,var)+m] = G[var, c, hc*128+p, m]
    g_pack = np.zeros((128, HC * C * 3 * 66), np.float32)
    for hc in range(HC):
        for c in range(C):
            for var in range(3):
                col = _g_col(hc, c, var)
                g_pack[:, col:col + 66] = G[var, c, hc * 128:(hc + 1) * 128, :]

    w1t = np.ascontiguousarray(W1.T).astype(np.float16)     # [E, H]
    g_pack = g_pack.astype(np.float16)
    b1c = np.ascontiguousarray(b1.reshape(HC, 128).T)       # [128, HC]
    brep = np.ascontiguousarray(np.broadcast_to(Bvec, (BL, SIG_LEN)))
    ident = np.eye(128, dtype=np.float32)

    # [B,C,T,E] -> per-core [TC, E, (c, tl, b)] fp16 (device reads xT directly)
    xs = x.reshape(N_CORES, BL, C, TC, TL, E).transpose(0, 3, 5, 2, 4, 1)
    xs = np.ascontiguousarray(xs.astype(np.float16)).reshape(
        N_CORES, TC, E, C * CW)
    return [
        {
            "x": xs[i],
            "w1t": w1t, "b1c": b1c, "g": g_pack,
            "brep": brep, "ident": ident,
        }
        for i in range(N_CORES)
    ]


def kernel(**inputs) -> np.ndarray:
    if "nc" not in _CACHE:
        _CACHE["nc"] = _build_bass()
    nc = _CACHE["nc"]

    in_maps = make_in_maps(inputs)
    res = run_bass_kernel_spmd(nc, in_maps, core_ids=list(range(N_CORES)))
    _CACHE["last_result"] = res
    y = np.concatenate([r["y"] for r in res.results], axis=0)   # [B, 1056]
    return y.reshape(B, 1, SIG_LEN).astype(np.float32)


if __name__ == "__main__":
    rng = np.random.default_rng(0)
    ins = {
        "encoder_output": rng.standard_normal((B, C, T, E), dtype=np.float32),
        "W1": rng.standard_normal((H, E), dtype=np.float32) / np.sqrt(E),
        "b1": rng.standard_normal((H,), dtype=np.float32) / np.sqrt(E),
        "W2": rng.standard_normal((E, H), dtype=np.float32) / np.sqrt(H),
        "b2": rng.standard_normal((E,), dtype=np.float32) / np.sqrt(H),
        "Winv": rng.standard_normal((SEG_LEN, E), dtype=np.float32) / np.sqrt(E),
        "binv": rng.standard_normal((SEG_LEN,), dtype=np.float32) / np.sqrt(E),
        "Wconv": rng.standard_normal((1, C, 3), dtype=np.float32) / np.sqrt(C * 3),
        "bconv": rng.standard_normal((1,), dtype=np.float32) / np.sqrt(C * 3),
    }
    out = kernel(**ins)
    print("kernel output", out.shape, out.dtype)


# revision 32
# speedup vs baseline: 1.6155x; 1.0026x over previous
"""Trainium2 Bass kernel for nn_Decoder (MLP -> inverse token embedding ->
overlap-add -> channel-merge conv), data-parallel over batch on 8 NeuronCores.

Self-contained: hardcodes shapes; host-side numpy folds everything after the
first Linear+ReLU into per-channel fused matrices G (W2 -> Winv -> overlap-add
normalization -> 3-tap channel conv), so the device pipeline is:

    xT[E,tok] (host pre-transposed, fp16) --matmul W1T--> h[Hc,tok] in PSUM
    --ACT/DVE relu+bias--> hT in SBUF --matmul G (accum over c,Hc)--> v[66,tok]
    --PE transpose--> vT[b,66] --strided adds (overlap-add)--> y[b,1056]

Sharding: batch 1024 -> 8 cores x 128.
"""

import numpy as np

import concourse.bacc as bacc
import concourse.mybir as mybir
from concourse.bass_utils import run_bass_kernel_spmd
from concourse.tile import TileContext

# problem shapes (hardcoded per contract)
B, C, T, E, H = 1024, 8, 32, 128, 256
SEG_LEN, SIG_LEN, NUM_SEG, STEP = 64, 1056, 32, 32
N_CORES = 8
BL = B // N_CORES          # local batch per core = 128
HC = H // 128              # H chunks = 2
TC = 8                     # t-chunks
TL = T // TC               # t per chunk = 4
CW = TL * 128              # columns per (c, t-chunk) = 512
FD = mybir.dt.float32
FR = mybir.dt.float32r   # fp32 storage, FP22 multiply: 4x faster PE
FH = mybir.dt.float16

_CACHE = {}


def _host_prep(W1, b1, W2, b2, Winv, binv, Wconv, bconv):
    """Fold W2/Winv/normalization/conv into G [3var][C][H,66] and bias B[1056]."""
    counter = np.zeros(SIG_LEN, np.float64)
    for t in range(NUM_SEG):
        counter[t * STEP: t * STEP + SEG_LEN] += 1.0
    n = 1.0 / counter

    F = Winv.astype(np.float64) @ W2.astype(np.float64)          # [64, H]
    binv2 = Winv.astype(np.float64) @ b2.astype(np.float64) + binv.astype(np.float64)
    Wc = Wconv[0].astype(np.float64)                             # [C, 3]

    def n_of(var, s):
        if var == 0:
            return n[s]
        if var == 2:
            return n[992 + s]
        return 0.5

    G = np.zeros((3, C, H, 66), np.float64)
    for var in range(3):
        for c in range(C):
            for m_idx in range(66):
                for k in range(3):
                    s = m_idx + k - 2
                    if 0 <= s < SEG_LEN:
                        G[var, c, :, m_idx] += Wc[c, k] * n_of(var, s) * F[s, :]

    sig_b = np.zeros(SIG_LEN, np.float64)
    for t in range(NUM_SEG):
        sig_b[t * STEP: t * STEP + SEG_LEN] += binv2
    sig_b *= n
    Bvec = np.full(SIG_LEN, float(np.asarray(bconv).reshape(-1)[0]), np.float64)
    q = np.arange(SIG_LEN)
    for k in range(3):
        qq = q + k - 1
        valid = (qq >= 0) & (qq < SIG_LEN)
        for c in range(C):
            Bvec[valid] += Wc[c, k] * sig_b[qq[valid]]
    return G.astype(np.float32), Bvec.astype(np.float32)


def _g_col(hc, c, var):
    """Column offset of G slice (hc, c, var) inside g_sb [128, 2*8*3*66]."""
    return ((hc * C + c) * 3 + var) * 66


def _build_bass():
    nc = bacc.Bacc("TRN2")

    # host pre-transposed to [TC, E, C*TL*BL] fp16: per t-chunk one contiguous
    # [128, 4096] block, columns ordered (c, tl, b)
    x = nc.dram_tensor("x", [TC, E, C * CW], FH, kind="ExternalInput")
    w1t = nc.dram_tensor("w1t", [E, H], FH, kind="ExternalInput")
    b1c = nc.dram_tensor("b1c", [128, HC], FD, kind="ExternalInput")
    g = nc.dram_tensor("g", [128, HC * C * 3 * 66], FH, kind="ExternalInput")
    brep = nc.dram_tensor("brep", [BL, SIG_LEN], FD, kind="ExternalInput")
    ident = nc.dram_tensor("ident", [128, 128], FR, kind="ExternalInput")
    y = nc.dram_tensor("y", [BL, SIG_LEN], FD, kind="ExternalOutput")

    with TileContext(nc) as tc:
        with (
            tc.tile_pool(name="consts", bufs=1) as consts,
            tc.tile_pool(name="xin", bufs=2) as xin_pool,
            tc.tile_pool(name="ht", bufs=2) as ht_pool,
            tc.tile_pool(name="vsb", bufs=3) as vsb_pool,
            tc.tile_pool(name="big", bufs=1) as big_pool,
            tc.tile_pool(name="pe_out", bufs=1, space="PSUM") as peout_pool,
            tc.tile_pool(name="h_ps", bufs=4, space="PSUM") as hps_pool,
            tc.tile_pool(name="v_ps", bufs=3, space="PSUM") as vps_pool,
        ):
            w1t_sb = consts.tile([E, H], FH)
            b1c_sb = consts.tile([128, HC], FD)
            g_sb = consts.tile([128, HC * C * 3 * 66], FH)
            ident_sb = consts.tile([128, 128], FR)
            warm_a = consts.tile([128, 128], FH)
            warm_b = consts.tile([128, 384], FH)
            brep_sb = big_pool.tile([BL, SIG_LEN], FD)

            V_sb = big_pool.tile([BL, T * 66], FD)      # v transposed: [b, t*66+m]
            y_sb = big_pool.tile([BL, SIG_LEN], FD)

            xin_tiles = {}

            def emit_x_load(tcix, split_first=False):
                xt = xin_pool.tile([E, C * CW], FH, tag="xin")
                if split_first:
                    # first chunk in 4 pieces: more DMA-queue share against
                    # the const loads, and c0-c1 land early so MLP1 starts
                    nc.sync.dma_start(out=xt[:, 0:2 * CW],
                                      in_=x[tcix, :, 0:2 * CW])
                    xin_tiles[tcix] = (xt, True)
                    return
                if tcix == 1:
                    # two halves so tcix1's first channels land sooner
                    nc.sync.dma_start(out=xt[:, 0:4 * CW],
                                      in_=x[tcix, :, 0:4 * CW])
                    xin_tiles[tcix] = (xt, True)
                    return
                nc.sync.dma_start(out=xt[:], in_=x[tcix, :, :])
                xin_tiles[tcix] = (xt, False)

            def emit_x_load_second_half(tcix):
                xt, _ = xin_tiles[tcix]
                nc.sync.dma_start(out=xt[:, 4 * CW:C * CW],
                                  in_=x[tcix, :, 4 * CW:C * CW])

            def emit_x_load_rest(tcix):
                xt, _ = xin_tiles[tcix]
                for k in range(1, 4):
                    nc.sync.dma_start(
                        out=xt[:, 2 * k * CW:2 * (k + 1) * CW],
                        in_=x[tcix, :, 2 * k * CW:2 * (k + 1) * CW])

            # PE warm-up: the HAM clock gate releases only after ~3.4us of
            # sustained PE activity; burn garbage matmuls during the DMA head
            # so real matmuls start at 2.4 GHz. Reuses an h_ps buffer (WAW on
            # the in-order PE queue, so no stall).
            def emit_warmup():
                warm_ps = hps_pool.tile([128, CW], FD, tag="h_ps",
                                        name="warm_ps")
                nc.gpsimd.memset(warm_a[:], 1.0)
                nc.gpsimd.memset(warm_b[:], 1.0)
                # trigger ACT_TABLE_LOAD now, not at the first real relu
                nc.scalar.activation(
                    warm_a[:], warm_a[:],
                    mybir.ActivationFunctionType.Relu, scale=1.0)
                for _ in range(14):
                    nc.tensor.matmul(
                        warm_ps[:, 0:384], warm_b[:, 0:128], warm_b[:],
                        start=True, stop=True)

            # software pipeline: fused stage runs one t-chunk behind MLP1
            ht_tiles = {}

            # greedy ACT/DVE load balancer for PSUM->SBUF copies and relus
            eng_busy = {"act": 0.0, "dve": 0.0}

            def pick_engine(fd):
                ca = (352 + fd) / 1.2
                cd = (151 + fd) / 0.96
                if eng_busy["act"] + ca <= eng_busy["dve"] + cd:
                    eng_busy["act"] += ca
                    return "act"
                eng_busy["dve"] += cd
                return "dve"

            def bal_copy(out, in_, fd):
                if pick_engine(fd) == "act":
                    nc.scalar.copy(out=out, in_=in_)
                else:
                    nc.vector.tensor_copy(out=out, in_=in_)

            def chunk_ranges(tcix):
                # column ranges with uniform G variant; cols = tl*128 + b
                if tcix == 0:
                    return [(0, 128, 0), (128, CW, 1)]        # t=0 -> var 0
                if tcix == TC - 1:
                    return [(0, 384, 1), (384, CW, 2)]        # t=31 -> var 2
                return [(0, CW, 1)]

            def emit_mlp1(tcix, c):
                """MLP1 for one channel: 2 matmuls (one per hc) into 1-bank
                PSUM tiles; bufs=4 gives two channels of pipeline depth so
                the drains never stall the PE."""
                ht = ht_tiles[tcix]
                xt, _ = xin_tiles[tcix]
                cp, half = divmod(c, 2)
                h_list = []
                for hc in range(HC):
                    h_ps = hps_pool.tile([128, CW], FD, tag="h_ps",
                                         name=f"h_ps_{tcix}_{c}_{hc}")
                    nc.tensor.matmul(
                        h_ps[:],
                        w1t_sb[:, hc * 128:(hc + 1) * 128],
                        xt[:, c * CW:(c + 1) * CW],
                        start=True, stop=True,
                    )
                    h_list.append(h_ps)
                for hc in range(HC):
                    dst = ht[(cp, hc)][:, half * CW:(half + 1) * CW]
                    src = h_list[hc][:]
                    if pick_engine(CW) == "act":
                        nc.scalar.activation(
                            dst, src,
                            mybir.ActivationFunctionType.Relu,
                            bias=b1c_sb[:, hc:hc + 1], scale=1.0,
                        )
                    else:
                        nc.vector.tensor_scalar(
                            dst, src,
                            b1c_sb[:, hc:hc + 1], 0.0,
                            mybir.AluOpType.add, mybir.AluOpType.max,
                        )

            def emit_fused(tcix, v_tiles, c):
                """fused G matmuls for channel c accumulating into v_tiles."""
                ht = ht_tiles[tcix]
                cp, half = divmod(c, 2)
                for (lo, hi, var, v_ps) in v_tiles:
                    for hc in range(HC):
                        i = c * HC + hc
                        nc.tensor.matmul(
                            v_ps[:, lo:hi],
                            g_sb[:, _g_col(hc, c, var):_g_col(hc, c, var) + 66],
                            ht[(cp, hc)][:, half * CW + lo:half * CW + hi],
                            start=(i == 0), stop=(i == C * HC - 1),
                        )

            def emit_vtrans(tcix, v_tiles):
                """copy v psum -> sbuf, PE-transpose per t into one PSUM tile,
                single merged copy into V_sb."""
                del ht_tiles[tcix]
                v_sb = vsb_pool.tile([66, CW], FR, tag="v_sb")
                for (lo, hi, var, v_ps) in v_tiles:
                    bal_copy(v_sb[:, lo:hi], v_ps[:, lo:hi], hi - lo)
                vt_ps = peout_pool.tile([128, 264], FR, tag="pe_out")
                for tl in range(TL):
                    nc.tensor.transpose(
                        vt_ps[:, tl * 66:(tl + 1) * 66],
                        v_sb[:, tl * 128:(tl + 1) * 128],
                        ident_sb[0:66, 0:66],
                    )
                bal_copy(V_sb[:, tcix * 264:(tcix + 1) * 264], vt_ps[:], 264)

            # overlap-add assembly in rounds (per watermark) so it overlaps
            # with later chunks instead of serializing at the end
            V3 = V_sb[:].rearrange("b (t m) -> b t m", m=66)
            Y3 = y_sb[:].rearrange("b (j r) -> b j r", r=32)
            B3 = brep_sb[:].rearrange("b (j r) -> b j r", r=32)

            def emit_y_assembly(j_lo, j_hi, eng):
                """Assemble y blocks j in [j_lo, j_hi); requires V[t] for
                t <= j_hi (uses t=j+1 for the r=31 edge)."""
                jm = min(j_hi, 32)      # main1 defined for j<=31
                if jm > j_lo:
                    eng.tensor_add(
                        out=Y3[:, j_lo:jm, :], in0=V3[:, j_lo:jm, 1:33],
                        in1=B3[:, j_lo:jm, :])
                if j_hi == 33:          # last block: bias only here
                    eng.tensor_copy(
                        out=y_sb[:, 1024:1056], in_=brep_sb[:, 1024:1056])
                lo = max(1, j_lo)
                if j_hi > lo:           # += v[:, j-1, r+33]
                    eng.tensor_add(
                        out=Y3[:, lo:j_hi, :], in0=Y3[:, lo:j_hi, :],
                        in1=V3[:, lo - 1:j_hi - 1, 33:65])
                lo = max(2, j_lo)
                if j_hi > lo:           # r=0: += v[:, j-2, 65]
                    eng.tensor_add(
                        out=Y3[:, lo:j_hi, 0], in0=Y3[:, lo:j_hi, 0],
                        in1=V3[:, lo - 2:j_hi - 2, 65])
                hi = min(j_hi, 31)
                if hi > j_lo:           # r=31: += v[:, j+1, 0]
                    eng.tensor_add(
                        out=Y3[:, j_lo:hi, 31], in0=Y3[:, j_lo:hi, 31],
                        in1=V3[:, j_lo + 1:hi + 1, 0])

            # after vtrans(i) V[t] is final for t <= 4i+3, so y blocks
            # j < min(4i+3, 33) can assemble (block j reads up to t=j+1)
            y_wm = [0]

            def emit_rounds(i):
                if i < TC - 1:
                    j_hi = min(4 * i + 3, 33)
                    if j_hi > y_wm[0]:
                        emit_y_assembly(y_wm[0], j_hi, nc.gpsimd)
                        y_wm[0] = j_hi
                else:
                    # final round: split across gpsimd + vector (independent
                    # j ranges) to shorten the tail
                    mid = (y_wm[0] + 33 + 1) // 2
                    emit_y_assembly(y_wm[0], mid, nc.gpsimd)
                    emit_y_assembly(mid, 33, nc.vector)
                    y_wm[0] = 33
                # progressive stores once column ranges are final (on the
                # gpsimd queue so their waits never block Sync x-dispatches)
                if y_wm[0] >= 15 and not store_done[0]:
                    nc.gpsimd.dma_start(out=y[:, 0:480], in_=y_sb[:, 0:480])
                    store_done[0] = True
                if y_wm[0] >= 27 and not store_done[1]:
                    nc.gpsimd.dma_start(out=y[:, 480:864], in_=y_sb[:, 480:864])
                    store_done[1] = True

            store_done = [False, False]

            prev = None          # (tcix, v_tiles) of the chunk awaiting fused
            emit_warmup()
            nc.sync.dma_start(out=b1c_sb[:], in_=b1c[:])
            emit_x_load(0, split_first=True)
            nc.sync.dma_start(out=w1t_sb[:], in_=w1t[:])
            emit_x_load_rest(0)
            emit_x_load(1)
            nc.sync.dma_start(out=g_sb[:], in_=g[:])
            emit_x_load_second_half(1)
            nc.sync.dma_start(out=ident_sb[:], in_=ident[:])
            nc.sync.dma_start(out=brep_sb[:], in_=brep[:])
            for tcix in range(TC):
                ht_tiles[tcix] = {
                    (cp, hc): ht_pool.tile(
                        [128, 2 * CW], FH,
                        tag=f"ht{hc}_{cp}", name=f"ht_{tcix}_{hc}_{cp}")
                    for cp in range(C // 2) for hc in range(HC)}
                if tcix + 2 < TC:
                    emit_x_load(tcix + 2)
                # interleave: MLP1(tcix, cp) with fused(tcix-1, c) so PE
                # always has matmul work while relu copies drain PSUM
                for c in range(C):
                    emit_mlp1(tcix, c)
                    if prev is not None:
                        emit_fused(prev[0], prev[1], c)
                if prev is not None:
                    emit_vtrans(prev[0], prev[1])
                    emit_rounds(prev[0])
                del xin_tiles[tcix]
                v_tiles = [
                    (lo, hi, var, vps_pool.tile(
                        [66, CW], FD, tag="v_ps", name=f"v_ps_{tcix}_{lo}"))
                    for (lo, hi, var) in chunk_ranges(tcix)]
                prev = (tcix, v_tiles)
            for c in range(C):
                emit_fused(prev[0], prev[1], c)
            emit_vtrans(prev[0], prev[1])
            emit_rounds(prev[0])

            nc.gpsimd.dma_start(out=y[:, 864:SIG_LEN], in_=y_sb[:, 864:SIG_LEN])

    nc.finalize()
    return nc


def make_in_maps(inputs):
    """Per-core input maps (shared by kernel(), sim checks, and bench)."""
    x = np.asarray(inputs["encoder_output"], dtype=np.float32)
    W1 = np.asarray(inputs["W1"], np.float32)
    b1 = np.asarray(inputs["b1"], np.float32)

    G, Bvec = _host_prep(
        inputs["W1"], inputs["b1"], inputs["W2"], inputs["b2"],
        inputs["Winv"], inputs["binv"], inputs["Wconv"], inputs["bconv"])

    # pack G -> [128, HC*C*3*66]: g_sb[p, _g_col(hc,c,var)+m] = G[var, c, hc*128+p, m]
    g_pack = np.zeros((128, HC * C * 3 * 66), np.float32)
    for hc in range(HC):
        for c in range(C):
            for var in range(3):
                col = _g_col(hc, c, var)
                g_pack[:, col:col + 66] = G[var, c, hc * 128:(hc + 1) * 128, :]

    w1t = np.ascontiguousarray(W1.T).astype(np.float16)     # [E, H]
    g_pack = g_pack.astype(np.float16)
    b1c = np.ascontiguousarray(b1.reshape(HC, 128).T)       # [128, HC]
    brep = np.ascontiguousarray(np.broadcast_to(Bvec, (BL, SIG_LEN)))
    ident = np.eye(128, dtype=np.float32)

    # [B,C,T,E] -> per-core [TC, E, (c, tl, b)] fp16 (device reads xT directly)
    xs = x.reshape(N_CORES, BL, C, TC, TL, E).transpose(0, 3, 5, 2, 4, 1)
    xs = np.ascontiguousarray(xs.astype(np.float16)).reshape(
        N_CORES, TC, E, C * CW)
    return [
        {
            "x": xs[i],
            "w1t": w1t, "b1c": b1c, "g": g_pack,
            "brep": brep, "ident": ident,
        }
        for i in range(N_CORES)
    ]


def kernel(**inputs) -> np.ndarray:
    if "nc" not in _CACHE:
        _CACHE["nc"] = _build_bass()
    nc = _CACHE["nc"]

    in_maps = make_in_maps(inputs)
    try:
        res = run_bass_kernel_spmd(nc, in_maps, core_ids=list(range(N_CORES)))
    except ModuleNotFoundError:
        # BASS_TRACE was set but the axon NTFF profile hook module is absent
        # in this environment; rerun without tracing.
        import os
        os.environ["BASS_NEVER_TRACE"] = "1"
        res = run_bass_kernel_spmd(nc, in_maps, core_ids=list(range(N_CORES)))
    _CACHE["last_result"] = res
    y = np.concatenate([r["y"] for r in res.results], axis=0)   # [B, 1056]
    return y.reshape(B, 1, SIG_LEN).astype(np.float32)


if __name__ == "__main__":
    rng = np.random.default_rng(0)
    ins = {
        "encoder_output": rng.standard_normal((B, C, T, E), dtype=np.float32),
        "W1": rng.standard_normal((H, E), dtype=np.float32) / np.sqrt(E),
        "b1": rng.standard_normal((H,), dtype=np.float32) / np.sqrt(E),
        "W2": rng.standard_normal((E, H), dtype=np.float32) / np.sqrt(H),
        "b2": rng.standard_normal((E,), dtype=np.float32) / np.sqrt(H),
        "Winv": rng.standard_normal((SEG_LEN, E), dtype=np.float32) / np.sqrt(E),
        "binv": rng.standard_normal((SEG_LEN,), dtype=np.float32) / np.sqrt(E),
        "Wconv": rng.standard_normal((1, C, 3), dtype=np.float32) / np.sqrt(C * 3),
        "bconv": rng.standard_normal((1,), dtype=np.float32) / np.sqrt(C * 3),
    }
    out = kernel(**ins)
    print("kernel output", out.shape, out.dtype)


# revision 40
# speedup vs baseline: 1.6349x; 1.0120x over previous
"""Trainium2 Bass kernel for nn_Decoder (MLP -> inverse token embedding ->
overlap-add -> channel-merge conv), data-parallel over batch on 8 NeuronCores.

Self-contained: hardcodes shapes; host-side numpy folds everything after the
first Linear+ReLU into per-channel fused matrices G (W2 -> Winv -> overlap-add
normalization -> 3-tap channel conv), so the device pipeline is:

    xT[E,tok] (host pre-transposed, fp16) --matmul W1T--> h[Hc,tok] in PSUM
    --ACT/DVE relu+bias--> hT in SBUF --matmul G (accum over c,Hc)--> v[66,tok]
    --PE transpose--> vT[b,66] --strided adds (overlap-add)--> y[b,1056]

Sharding: batch 1024 -> 8 cores x 128.
"""

import numpy as np

import concourse.bacc as bacc
import concourse.mybir as mybir
from concourse.bass_utils import run_bass_kernel_spmd
from concourse.tile import TileContext

# problem shapes (hardcoded per contract)
B, C, T, E, H = 1024, 8, 32, 128, 256
SEG_LEN, SIG_LEN, NUM_SEG, STEP = 64, 1056, 32, 32
N_CORES = 8
BL = B // N_CORES          # local batch per core = 128
HC = H // 128              # H chunks = 2
TC = 8                     # t-chunks
TL = T // TC               # t per chunk = 4
CW = TL * 128              # columns per (c, t-chunk) = 512
FD = mybir.dt.float32
FR = mybir.dt.float32r   # fp32 storage, FP22 multiply: 4x faster PE
FH = mybir.dt.float16

_CACHE = {}


def _host_prep(W1, b1, W2, b2, Winv, binv, Wconv, bconv):
    """Fold W2/Winv/normalization/conv into G [3var][C][H,66] and bias B[1056]."""
    counter = np.zeros(SIG_LEN, np.float64)
    for t in range(NUM_SEG):
        counter[t * STEP: t * STEP + SEG_LEN] += 1.0
    n = 1.0 / counter

    F = Winv.astype(np.float64) @ W2.astype(np.float64)          # [64, H]
    binv2 = Winv.astype(np.float64) @ b2.astype(np.float64) + binv.astype(np.float64)
    Wc = Wconv[0].astype(np.float64)                             # [C, 3]

    def n_of(var, s):
        if var == 0:
            return n[s]
        if var == 2:
            return n[992 + s]
        return 0.5

    G = np.zeros((3, C, H, 66), np.float64)
    for var in range(3):
        for c in range(C):
            for m_idx in range(66):
                for k in range(3):
                    s = m_idx + k - 2
                    if 0 <= s < SEG_LEN:
                        G[var, c, :, m_idx] += Wc[c, k] * n_of(var, s) * F[s, :]

    sig_b = np.zeros(SIG_LEN, np.float64)
    for t in range(NUM_SEG):
        sig_b[t * STEP: t * STEP + SEG_LEN] += binv2
    sig_b *= n
    Bvec = np.full(SIG_LEN, float(np.asarray(bconv).reshape(-1)[0]), np.float64)
    q = np.arange(SIG_LEN)
    for k in range(3):
        qq = q + k - 1
        valid = (qq >= 0) & (qq < SIG_LEN)
        for c in range(C):
            Bvec[valid] += Wc[c, k] * sig_b[qq[valid]]
    return G.astype(np.float32), Bvec.astype(np.float32)


def _g_col(hc, c, var):
    """Column offset of G slice (hc, c, var) inside g_sb [128, 2*8*3*66]."""
    return ((hc * C + c) * 3 + var) * 66


def _build_bass():
    nc = bacc.Bacc("TRN2")

    # host pre-transposed to [TC, E, C*TL*BL] fp16: per t-chunk one contiguous
    # [128, 4096] block, columns ordered (c, tl, b)
    x = nc.dram_tensor("x", [TC, E, C * CW], FH, kind="ExternalInput")
    w1t = nc.dram_tensor("w1t", [E, H], FH, kind="ExternalInput")
    b1c = nc.dram_tensor("b1c", [128, HC], FD, kind="ExternalInput")
    g = nc.dram_tensor("g", [128, HC * C * 3 * 66], FH, kind="ExternalInput")
    brep = nc.dram_tensor("brep", [BL, SIG_LEN], FD, kind="ExternalInput")
    ident = nc.dram_tensor("ident", [128, 128], FH, kind="ExternalInput")
    y = nc.dram_tensor("y", [BL, SIG_LEN], FD, kind="ExternalOutput")

    with TileContext(nc) as tc:
        with (
            tc.tile_pool(name="consts", bufs=1) as consts,
            tc.tile_pool(name="xin", bufs=2) as xin_pool,
            tc.tile_pool(name="ht", bufs=2) as ht_pool,
            tc.tile_pool(name="vsb", bufs=3) as vsb_pool,
            tc.tile_pool(name="big", bufs=1) as big_pool,
            tc.tile_pool(name="pe_out", bufs=1, space="PSUM") as peout_pool,
            tc.tile_pool(name="h_ps", bufs=4, space="PSUM") as hps_pool,
            tc.tile_pool(name="v_ps", bufs=3, space="PSUM") as vps_pool,
        ):
            w1t_sb = consts.tile([E, H], FH)
            b1c_sb = consts.tile([128, HC], FD)
            g_sb = consts.tile([128, HC * C * 3 * 66], FH)
            ident_sb = consts.tile([128, 128], FH)
            warm_a = consts.tile([128, 128], FH)
            warm_b = consts.tile([128, 384], FH)
            brep_sb = big_pool.tile([BL, SIG_LEN], FD)

            V_sb = big_pool.tile([BL, T * 66], FD)      # v transposed: [b, t*66+m]
            y_sb = big_pool.tile([BL, SIG_LEN], FD)

            xin_tiles = {}

            def emit_x_load(tcix, split_first=False):
                xt = xin_pool.tile([E, C * CW], FH, tag="xin")
                if split_first:
                    # first chunk in 4 pieces: more DMA-queue share against
                    # the const loads, and c0-c1 land early so MLP1 starts
                    nc.sync.dma_start(out=xt[:, 0:2 * CW],
                                      in_=x[tcix, :, 0:2 * CW])
                    xin_tiles[tcix] = (xt, True)
                    return
                if tcix == 1:
                    # first quarter separately so tcix1's c0-c1 land sooner
                    nc.sync.dma_start(out=xt[:, 0:2 * CW],
                                      in_=x[tcix, :, 0:2 * CW])
                    nc.sync.dma_start(out=xt[:, 2 * CW:4 * CW],
                                      in_=x[tcix, :, 2 * CW:4 * CW])
                    xin_tiles[tcix] = (xt, True)
                    return
                nc.sync.dma_start(out=xt[:], in_=x[tcix, :, :])
                xin_tiles[tcix] = (xt, False)

            def emit_x_load_second_half(tcix):
                xt, _ = xin_tiles[tcix]
                nc.sync.dma_start(out=xt[:, 4 * CW:C * CW],
                                  in_=x[tcix, :, 4 * CW:C * CW])

            def emit_x_load_rest(tcix):
                xt, _ = xin_tiles[tcix]
                for k in range(1, 4):
                    nc.sync.dma_start(
                        out=xt[:, 2 * k * CW:2 * (k + 1) * CW],
                        in_=x[tcix, :, 2 * k * CW:2 * (k + 1) * CW])

            # PE warm-up: the HAM clock gate releases only after ~3.4us of
            # sustained PE activity; burn garbage matmuls during the DMA head
            # so real matmuls start at 2.4 GHz. Reuses an h_ps buffer (WAW on
            # the in-order PE queue, so no stall).
            def emit_warmup():
                warm_ps = hps_pool.tile([128, CW], FD, tag="h_ps",
                                        name="warm_ps")
                nc.gpsimd.memset(warm_a[:], 1.0)
                nc.gpsimd.memset(warm_b[:], 1.0)
                # trigger ACT_TABLE_LOAD now, not at the first real relu
                nc.scalar.activation(
                    warm_a[:], warm_a[:],
                    mybir.ActivationFunctionType.Relu, scale=1.0)
                for _ in range(14):
                    nc.tensor.matmul(
                        warm_ps[:, 0:384], warm_b[:, 0:128], warm_b[:],
                        start=True, stop=True)

            # software pipeline: fused stage runs one t-chunk behind MLP1
            ht_tiles = {}

            # greedy ACT/DVE load balancer for PSUM->SBUF copies and relus
            eng_busy = {"act": 0.0, "dve": 0.0}

            def pick_engine(fd):
                ca = (352 + fd) / 1.2
                cd = (151 + fd) / 0.96
                if eng_busy["act"] + ca <= eng_busy["dve"] + cd:
                    eng_busy["act"] += ca
                    return "act"
                eng_busy["dve"] += cd
                return "dve"

            def bal_copy(out, in_, fd):
                if pick_engine(fd) == "act":
                    nc.scalar.copy(out=out, in_=in_)
                else:
                    nc.vector.tensor_copy(out=out, in_=in_)

            def chunk_ranges(tcix):
                # column ranges with uniform G variant; cols = tl*128 + b
                if tcix == 0:
                    return [(0, 128, 0), (128, CW, 1)]        # t=0 -> var 0
                if tcix == TC - 1:
                    return [(0, 384, 1), (384, CW, 2)]        # t=31 -> var 2
                return [(0, CW, 1)]

            def emit_mlp1(tcix, c):
                """MLP1 for one channel: 2 matmuls (one per hc) into 1-bank
                PSUM tiles; bufs=4 gives two channels of pipeline depth so
                the drains never stall the PE."""
                ht = ht_tiles[tcix]
                xt, _ = xin_tiles[tcix]
                cp, half = divmod(c, 2)
                h_list = []
                for hc in range(HC):
                    h_ps = hps_pool.tile([128, CW], FD, tag="h_ps",
                                         name=f"h_ps_{tcix}_{c}_{hc}")
                    nc.tensor.matmul(
                        h_ps[:],
                        w1t_sb[:, hc * 128:(hc + 1) * 128],
                        xt[:, c * CW:(c + 1) * CW],
                        start=True, stop=True,
                    )
                    h_list.append(h_ps)
                for hc in range(HC):
                    dst = ht[(cp, hc)][:, half * CW:(half + 1) * CW]
                    src = h_list[hc][:]
                    if pick_engine(CW) == "act":
                        nc.scalar.activation(
                            dst, src,
                            mybir.ActivationFunctionType.Relu,
                            bias=b1c_sb[:, hc:hc + 1], scale=1.0,
                        )
                    else:
                        nc.vector.tensor_scalar(
                            dst, src,
                            b1c_sb[:, hc:hc + 1], 0.0,
                            mybir.AluOpType.add, mybir.AluOpType.max,
                        )

            def emit_fused(tcix, v_tiles, c):
                """fused G matmuls for channel c accumulating into v_tiles."""
                ht = ht_tiles[tcix]
                cp, half = divmod(c, 2)
                for (lo, hi, var, v_ps) in v_tiles:
                    for hc in range(HC):
                        i = c * HC + hc
                        nc.tensor.matmul(
                            v_ps[:, lo:hi],
                            g_sb[:, _g_col(hc, c, var):_g_col(hc, c, var) + 66],
                            ht[(cp, hc)][:, half * CW + lo:half * CW + hi],
                            start=(i == 0), stop=(i == C * HC - 1),
                        )

            def emit_vtrans(tcix, v_tiles):
                """copy v psum -> sbuf, PE-transpose per t into one PSUM tile,
                single merged copy into V_sb."""
                del ht_tiles[tcix]
                v_sb = vsb_pool.tile([66, CW], FH, tag="v_sb")
                for (lo, hi, var, v_ps) in v_tiles:
                    bal_copy(v_sb[:, lo:hi], v_ps[:, lo:hi], hi - lo)
                vt_ps = peout_pool.tile([128, 264], FH, tag="pe_out")
                for tl in range(TL):
                    nc.tensor.transpose(
                        vt_ps[:, tl * 66:(tl + 1) * 66],
                        v_sb[:, tl * 128:(tl + 1) * 128],
                        ident_sb[0:66, 0:66],
                    )
                bal_copy(V_sb[:, tcix * 264:(tcix + 1) * 264], vt_ps[:], 264)

            # overlap-add assembly in rounds (per watermark) so it overlaps
            # with later chunks instead of serializing at the end
            V3 = V_sb[:].rearrange("b (t m) -> b t m", m=66)
            Y3 = y_sb[:].rearrange("b (j r) -> b j r", r=32)
            B3 = brep_sb[:].rearrange("b (j r) -> b j r", r=32)

            def emit_y_assembly(j_lo, j_hi, eng):
                """Assemble y blocks j in [j_lo, j_hi); requires V[t] for
                t <= j_hi (uses t=j+1 for the r=31 edge)."""
                jm = min(j_hi, 32)      # main1 defined for j<=31
                if jm > j_lo:
                    eng.tensor_add(
                        out=Y3[:, j_lo:jm, :], in0=V3[:, j_lo:jm, 1:33],
                        in1=B3[:, j_lo:jm, :])
                if j_hi == 33:          # last block: bias only here
                    eng.tensor_copy(
                        out=y_sb[:, 1024:1056], in_=brep_sb[:, 1024:1056])
                lo = max(1, j_lo)
                if j_hi > lo:           # += v[:, j-1, r+33]
                    eng.tensor_add(
                        out=Y3[:, lo:j_hi, :], in0=Y3[:, lo:j_hi, :],
                        in1=V3[:, lo - 1:j_hi - 1, 33:65])
                lo = max(2, j_lo)
                if j_hi > lo:           # r=0: += v[:, j-2, 65]
                    eng.tensor_add(
                        out=Y3[:, lo:j_hi, 0], in0=Y3[:, lo:j_hi, 0],
                        in1=V3[:, lo - 2:j_hi - 2, 65])
                hi = min(j_hi, 31)
                if hi > j_lo:           # r=31: += v[:, j+1, 0]
                    eng.tensor_add(
                        out=Y3[:, j_lo:hi, 31], in0=Y3[:, j_lo:hi, 31],
                        in1=V3[:, j_lo + 1:hi + 1, 0])

            # after vtrans(i) V[t] is final for t <= 4i+3, so y blocks
            # j < min(4i+3, 33) can assemble (block j reads up to t=j+1)
            y_wm = [0]

            def emit_rounds(i):
                if i < TC - 1:
                    j_hi = min(4 * i + 3, 33)
                    if j_hi > y_wm[0]:
                        emit_y_assembly(y_wm[0], j_hi, nc.gpsimd)
                        y_wm[0] = j_hi
                else:
                    # final round: split across gpsimd + vector (independent
                    # j ranges) to shorten the tail, each half stored as soon
                    # as it is assembled
                    mid = (y_wm[0] + 33 + 1) // 2
                    emit_y_assembly(y_wm[0], mid, nc.gpsimd)
                    nc.gpsimd.dma_start(out=y[:, 864:32 * mid],
                                        in_=y_sb[:, 864:32 * mid])
                    emit_y_assembly(mid, 33, nc.vector)
                    y_wm[0] = 33
                # progressive stores once column ranges are final (on the
                # gpsimd queue so their waits never block Sync x-dispatches)
                if y_wm[0] >= 15 and not store_done[0]:
                    nc.gpsimd.dma_start(out=y[:, 0:480], in_=y_sb[:, 0:480])
                    store_done[0] = True
                if y_wm[0] >= 27 and not store_done[1]:
                    nc.gpsimd.dma_start(out=y[:, 480:864], in_=y_sb[:, 480:864])
                    store_done[1] = True

            store_done = [False, False]

            prev = None          # (tcix, v_tiles) of the chunk awaiting fused
            emit_warmup()
            nc.sync.dma_start(out=b1c_sb[:], in_=b1c[:])
            emit_x_load(0, split_first=True)
            nc.sync.dma_start(out=w1t_sb[:], in_=w1t[:])
            emit_x_load_rest(0)
            emit_x_load(1)
            nc.sync.dma_start(out=g_sb[:], in_=g[:])
            emit_x_load_second_half(1)
            nc.sync.dma_start(out=ident_sb[:], in_=ident[:])
            nc.sync.dma_start(out=brep_sb[:], in_=brep[:])
            for tcix in range(TC):
                ht_tiles[tcix] = {
                    (cp, hc): ht_pool.tile(
                        [128, 2 * CW], FH,
                        tag=f"ht{hc}_{cp}", name=f"ht_{tcix}_{hc}_{cp}")
                    for cp in range(C // 2) for hc in range(HC)}
                if tcix + 2 < TC:
                    emit_x_load(tcix + 2)
                # interleave: MLP1(tcix, cp) with fused(tcix-1, c) so PE
                # always has matmul work while relu copies drain PSUM
                for c in range(C):
                    emit_mlp1(tcix, c)
                    if prev is not None:
                        emit_fused(prev[0], prev[1], c)
                if prev is not None:
                    emit_vtrans(prev[0], prev[1])
                    emit_rounds(prev[0])
                del xin_tiles[tcix]
                v_tiles = [
                    (lo, hi, var, vps_pool.tile(
                        [66, CW], FD, tag="v_ps", name=f"v_ps_{tcix}_{lo}"))
                    for (lo, hi, var) in chunk_ranges(tcix)]
                prev = (tcix, v_tiles)
            for c in range(C):
                emit_fused(prev[0], prev[1], c)
            emit_vtrans(prev[0], prev[1])
            emit_rounds(prev[0])

            mid_col = 32 * ((27 + 33 + 1) // 2)
            nc.sync.dma_start(out=y[:, mid_col:SIG_LEN],
                              in_=y_sb[:, mid_col:SIG_LEN])

    nc.finalize()
    return nc


def make_in_maps(inputs):
    """Per-core input maps (shared by kernel(), sim checks, and bench)."""
    x = np.asarray(inputs["encoder_output"], dtype=np.float32)
    W1 = np.asarray(inputs["W1"], np.float32)
    b1 = np.asarray(inputs["b1"], np.float32)

    G, Bvec = _host_prep(
        inputs["W1"], inputs["b1"], inputs["W2"], inputs["b2"],
        inputs["Winv"], inputs["binv"], inputs["Wconv"], inputs["bconv"])

    # pack G -> [128, HC*C*3*66]: g_sb[p, _g_col(hc,c,var)+m] = G[var, c, hc*128+p, m]
    g_pack = np.zeros((128, HC * C * 3 * 66), np.float32)
    for hc in range(HC):
        for c in range(C):
            for var in range(3):
                col = _g_col(hc, c, var)
                g_pack[:, col:col + 66] = G[var, c, hc * 128:(hc + 1) * 128, :]

    w1t = np.ascontiguousarray(W1.T).astype(np.float16)     # [E, H]
    g_pack = g_pack.astype(np.float16)
    b1c = np.ascontiguousarray(b1.reshape(HC, 128).T)       # [128, HC]
    brep = np.ascontiguousarray(np.broadcast_to(Bvec, (BL, SIG_LEN)))
    ident = np.eye(128, dtype=np.float16)

    # [B,C,T,E] -> per-core [TC, E, (c, tl, b)] fp16 (device reads xT directly)
    xs = x.reshape(N_CORES, BL, C, TC, TL, E).transpose(0, 3, 5, 2, 4, 1)
    xs = np.ascontiguousarray(xs.astype(np.float16)).reshape(
        N_CORES, TC, E, C * CW)
    return [
        {
            "x": xs[i],
            "w1t": w1t, "b1c": b1c, "g": g_pack,
            "brep": brep, "ident": ident,
        }
        for i in range(N_CORES)
    ]


def kernel(**inputs) -> np.ndarray:
    if "nc" not in _CACHE:
        _CACHE["nc"] = _build_bass()
    nc = _CACHE["nc"]

    in_maps = make_in_maps(inputs)
    try:
        res = run_bass_kernel_spmd(nc, in_maps, core_ids=list(range(N_CORES)))
    except ModuleNotFoundError:
        # BASS_TRACE was set but the axon NTFF profile hook module is absent
        # in this environment; rerun without tracing.
        import os
        os.environ["BASS_NEVER_TRACE"] = "1"
        res = run_bass_kernel_spmd(nc, in_maps, core_ids=list(range(N_CORES)))
    _CACHE["last_result"] = res
    y = np.concatenate([r["y"] for r in res.results], axis=0)   # [B, 1056]
    return y.reshape(B, 1, SIG_LEN).astype(np.float32)


if __name__ == "__main__":
    rng = np.random.default_rng(0)
    ins = {
        "encoder_output": rng.standard_normal((B, C, T, E), dtype=np.float32),
        "W1": rng.standard_normal((H, E), dtype=np.float32) / np.sqrt(E),
        "b1": rng.standard_normal((H,), dtype=np.float32) / np.sqrt(E),
        "W2": rng.standard_normal((E, H), dtype=np.float32) / np.sqrt(H),
        "b2": rng.standard_normal((E,), dtype=np.float32) / np.sqrt(H),
        "Winv": rng.standard_normal((SEG_LEN, E), dtype=np.float32) / np.sqrt(E),
        "binv": rng.standard_normal((SEG_LEN,), dtype=np.float32) / np.sqrt(E),
        "Wconv": rng.standard_normal((1, C, 3), dtype=np.float32) / np.sqrt(C * 3),
        "bconv": rng.standard_normal((1,), dtype=np.float32) / np.sqrt(C * 3),
    }
    out = kernel(**ins)
    print("kernel output", out.shape, out.dtype)


# revision 45
# speedup vs baseline: 1.6427x; 1.0048x over previous
"""Trainium2 Bass kernel for nn_Decoder (MLP -> inverse token embedding ->
overlap-add -> channel-merge conv), data-parallel over batch on 8 NeuronCores.

Self-contained: hardcodes shapes; host-side numpy folds everything after the
first Linear+ReLU into per-channel fused matrices G (W2 -> Winv -> overlap-add
normalization -> 3-tap channel conv), so the device pipeline is:

    xT[E,tok] (host pre-transposed, fp16) --matmul W1T--> h[Hc,tok] in PSUM
    --ACT/DVE relu+bias--> hT in SBUF --matmul G (accum over c,Hc)--> v[66,tok]
    --PE transpose--> vT[b,66] --strided adds (overlap-add)--> y[b,1056]

Sharding: batch 1024 -> 8 cores x 128.
"""

import numpy as np

import concourse.bacc as bacc
import concourse.mybir as mybir
from concourse.bass_utils import run_bass_kernel_spmd
from concourse.tile import TileContext

# problem shapes (hardcoded per contract)
B, C, T, E, H = 1024, 8, 32, 128, 256
SEG_LEN, SIG_LEN, NUM_SEG, STEP = 64, 1056, 32, 32
N_CORES = 8
BL = B // N_CORES          # local batch per core = 128
HC = H // 128              # H chunks = 2
TC = 8                     # t-chunks
TL = T // TC               # t per chunk = 4
CW = TL * 128              # columns per (c, t-chunk) = 512
FD = mybir.dt.float32
FR = mybir.dt.float32r   # fp32 storage, FP22 multiply: 4x faster PE
FH = mybir.dt.float16

_CACHE = {}


def _host_prep(W1, b1, W2, b2, Winv, binv, Wconv, bconv):
    """Fold W2/Winv/normalization/conv into G [3var][C][H,66] and bias B[1056]."""
    counter = np.zeros(SIG_LEN, np.float64)
    for t in range(NUM_SEG):
        counter[t * STEP: t * STEP + SEG_LEN] += 1.0
    n = 1.0 / counter

    F = Winv.astype(np.float64) @ W2.astype(np.float64)          # [64, H]
    binv2 = Winv.astype(np.float64) @ b2.astype(np.float64) + binv.astype(np.float64)
    Wc = Wconv[0].astype(np.float64)                             # [C, 3]

    def n_of(var, s):
        if var == 0:
            return n[s]
        if var == 2:
            return n[992 + s]
        return 0.5

    G = np.zeros((3, C, H, 66), np.float64)
    for var in range(3):
        for c in range(C):
            for m_idx in range(66):
                for k in range(3):
                    s = m_idx + k - 2
                    if 0 <= s < SEG_LEN:
                        G[var, c, :, m_idx] += Wc[c, k] * n_of(var, s) * F[s, :]

    sig_b = np.zeros(SIG_LEN, np.float64)
    for t in range(NUM_SEG):
        sig_b[t * STEP: t * STEP + SEG_LEN] += binv2
    sig_b *= n
    Bvec = np.full(SIG_LEN, float(np.asarray(bconv).reshape(-1)[0]), np.float64)
    q = np.arange(SIG_LEN)
    for k in range(3):
        qq = q + k - 1
        valid = (qq >= 0) & (qq < SIG_LEN)
        for c in range(C):
            Bvec[valid] += Wc[c, k] * sig_b[qq[valid]]
    return G.astype(np.float32), Bvec.astype(np.float32)


def _g_col(hc, c, var):
    """Column offset of G slice (hc, c, var) inside g_sb [128, 2*8*3*66]."""
    return ((hc * C + c) * 3 + var) * 66


def _build_bass():
    nc = bacc.Bacc("TRN2")

    # host pre-transposed to [TC, E, C*TL*BL] fp16: per t-chunk one contiguous
    # [128, 4096] block, columns ordered (c, tl, b)
    x = nc.dram_tensor("x", [TC, E, C * CW], FH, kind="ExternalInput")
    w1t = nc.dram_tensor("w1t", [E, H], FH, kind="ExternalInput")
    b1c = nc.dram_tensor("b1c", [128, HC], FD, kind="ExternalInput")
    g = nc.dram_tensor("g", [128, HC * C * 3 * 66], FH, kind="ExternalInput")
    bvec = nc.dram_tensor("bvec", [1, SIG_LEN], FD, kind="ExternalInput")
    ident = nc.dram_tensor("ident", [128, 128], FH, kind="ExternalInput")
    y = nc.dram_tensor("y", [BL, SIG_LEN], FD, kind="ExternalOutput")

    with TileContext(nc) as tc:
        with (
            tc.tile_pool(name="consts", bufs=1) as consts,
            tc.tile_pool(name="xin", bufs=2) as xin_pool,
            tc.tile_pool(name="ht", bufs=2) as ht_pool,
            tc.tile_pool(name="vsb", bufs=3) as vsb_pool,
            tc.tile_pool(name="big", bufs=1) as big_pool,
            tc.tile_pool(name="pe_out", bufs=1, space="PSUM") as peout_pool,
            tc.tile_pool(name="h_ps", bufs=4, space="PSUM") as hps_pool,
            tc.tile_pool(name="v_ps", bufs=3, space="PSUM") as vps_pool,
        ):
            w1t_sb = consts.tile([E, H], FH)
            b1c_sb = consts.tile([128, HC], FD)
            g_sb = consts.tile([128, HC * C * 3 * 66], FH)
            ident_sb = consts.tile([128, 128], FH)
            warm_a = consts.tile([128, 128], FH)
            warm_b = consts.tile([128, 384], FH)
            bvec_sb = consts.tile([1, SIG_LEN], FD)
            brep_sb = big_pool.tile([BL, SIG_LEN], FD)

            V_sb = big_pool.tile([BL, T * 66], FD)      # v transposed: [b, t*66+m]
            y_sb = big_pool.tile([BL, SIG_LEN], FD)

            xin_tiles = {}

            def emit_x_load(tcix, split_first=False):
                xt = xin_pool.tile([E, C * CW], FH, tag="xin")
                if split_first:
                    # first chunk in 4 pieces: more DMA-queue share against
                    # the const loads, and c0-c1 land early so MLP1 starts
                    nc.sync.dma_start(out=xt[:, 0:2 * CW],
                                      in_=x[tcix, :, 0:2 * CW])
                    xin_tiles[tcix] = (xt, True)
                    return
                if tcix == 1:
                    # first quarter separately so tcix1's c0-c1 land sooner
                    nc.sync.dma_start(out=xt[:, 0:2 * CW],
                                      in_=x[tcix, :, 0:2 * CW])
                    nc.sync.dma_start(out=xt[:, 2 * CW:4 * CW],
                                      in_=x[tcix, :, 2 * CW:4 * CW])
                    xin_tiles[tcix] = (xt, True)
                    return
                nc.sync.dma_start(out=xt[:], in_=x[tcix, :, :])
                xin_tiles[tcix] = (xt, False)

            def emit_x_load_second_half(tcix):
                xt, _ = xin_tiles[tcix]
                nc.sync.dma_start(out=xt[:, 4 * CW:C * CW],
                                  in_=x[tcix, :, 4 * CW:C * CW])

            def emit_x_load_rest(tcix):
                xt, _ = xin_tiles[tcix]
                for k in range(1, 4):
                    nc.sync.dma_start(
                        out=xt[:, 2 * k * CW:2 * (k + 1) * CW],
                        in_=x[tcix, :, 2 * k * CW:2 * (k + 1) * CW])

            # PE warm-up: the HAM clock gate releases only after ~3.4us of
            # sustained PE activity; burn garbage matmuls during the DMA head
            # so real matmuls start at 2.4 GHz. Reuses an h_ps buffer (WAW on
            # the in-order PE queue, so no stall).
            def emit_warmup():
                warm_ps = hps_pool.tile([128, CW], FD, tag="h_ps",
                                        name="warm_ps")
                nc.gpsimd.memset(warm_a[:], 1.0)
                nc.gpsimd.memset(warm_b[:], 1.0)
                # trigger ACT_TABLE_LOAD now, not at the first real relu
                nc.scalar.activation(
                    warm_a[:], warm_a[:],
                    mybir.ActivationFunctionType.Relu, scale=1.0)
                for _ in range(14):
                    nc.tensor.matmul(
                        warm_ps[:, 0:384], warm_b[:, 0:128], warm_b[:],
                        start=True, stop=True)

            # software pipeline: fused stage runs one t-chunk behind MLP1
            ht_tiles = {}

            # greedy ACT/DVE load balancer for PSUM->SBUF copies and relus
            eng_busy = {"act": 0.0, "dve": 0.0}

            def pick_engine(fd):
                ca = (352 + fd) / 1.2
                cd = (151 + fd) / 0.96
                if eng_busy["act"] + ca <= eng_busy["dve"] + cd:
                    eng_busy["act"] += ca
                    return "act"
                eng_busy["dve"] += cd
                return "dve"

            def bal_copy(out, in_, fd):
                if pick_engine(fd) == "act":
                    nc.scalar.copy(out=out, in_=in_)
                else:
                    nc.vector.tensor_copy(out=out, in_=in_)

            def chunk_ranges(tcix):
                # column ranges with uniform G variant; cols = tl*128 + b
                if tcix == 0:
                    return [(0, 128, 0), (128, CW, 1)]        # t=0 -> var 0
                if tcix == TC - 1:
                    return [(0, 384, 1), (384, CW, 2)]        # t=31 -> var 2
                return [(0, CW, 1)]

            def emit_mlp1(tcix, c):
                """MLP1 for one channel: 2 matmuls (one per hc) into 1-bank
                PSUM tiles; bufs=4 gives two channels of pipeline depth so
                the drains never stall the PE."""
                ht = ht_tiles[tcix]
                xt, _ = xin_tiles[tcix]
                cp, half = divmod(c, 2)
                h_list = []
                for hc in range(HC):
                    h_ps = hps_pool.tile([128, CW], FD, tag="h_ps",
                                         name=f"h_ps_{tcix}_{c}_{hc}")
                    nc.tensor.matmul(
                        h_ps[:],
                        w1t_sb[:, hc * 128:(hc + 1) * 128],
                        xt[:, c * CW:(c + 1) * CW],
                        start=True, stop=True,
                    )
                    h_list.append(h_ps)
                for hc in range(HC):
                    dst = ht[(cp, hc)][:, half * CW:(half + 1) * CW]
                    src = h_list[hc][:]
                    if pick_engine(CW) == "act":
                        nc.scalar.activation(
                            dst, src,
                            mybir.ActivationFunctionType.Relu,
                            bias=b1c_sb[:, hc:hc + 1], scale=1.0,
                        )
                    else:
                        nc.vector.tensor_scalar(
                            dst, src,
                            b1c_sb[:, hc:hc + 1], 0.0,
                            mybir.AluOpType.add, mybir.AluOpType.max,
                        )

            def emit_fused(tcix, v_tiles, c):
                """fused G matmuls for channel c accumulating into v_tiles."""
                ht = ht_tiles[tcix]
                cp, half = divmod(c, 2)
                for (lo, hi, var, v_ps) in v_tiles:
                    for hc in range(HC):
                        i = c * HC + hc
                        nc.tensor.matmul(
                            v_ps[:, lo:hi],
                            g_sb[:, _g_col(hc, c, var):_g_col(hc, c, var) + 66],
                            ht[(cp, hc)][:, half * CW + lo:half * CW + hi],
                            start=(i == 0), stop=(i == C * HC - 1),
                        )

            def emit_vtrans(tcix, v_tiles):
                """copy v psum -> sbuf, PE-transpose per t into one PSUM tile,
                single merged copy into V_sb."""
                del ht_tiles[tcix]
                v_sb = vsb_pool.tile([66, CW], FH, tag="v_sb")
                for (lo, hi, var, v_ps) in v_tiles:
                    bal_copy(v_sb[:, lo:hi], v_ps[:, lo:hi], hi - lo)
                vt_ps = peout_pool.tile([128, 264], FH, tag="pe_out")
                for tl in range(TL):
                    nc.tensor.transpose(
                        vt_ps[:, tl * 66:(tl + 1) * 66],
                        v_sb[:, tl * 128:(tl + 1) * 128],
                        ident_sb[0:66, 0:66],
                    )
                bal_copy(V_sb[:, tcix * 264:(tcix + 1) * 264], vt_ps[:], 264)

            # overlap-add assembly in rounds (per watermark) so it overlaps
            # with later chunks instead of serializing at the end
            V3 = V_sb[:].rearrange("b (t m) -> b t m", m=66)
            Y3 = y_sb[:].rearrange("b (j r) -> b j r", r=32)
            B3 = brep_sb[:].rearrange("b (j r) -> b j r", r=32)

            def emit_y_assembly(j_lo, j_hi, eng):
                """Assemble y blocks j in [j_lo, j_hi); requires V[t] for
                t <= j_hi (uses t=j+1 for the r=31 edge)."""
                jm = min(j_hi, 32)      # main1 defined for j<=31
                if jm > j_lo:
                    eng.tensor_add(
                        out=Y3[:, j_lo:jm, :], in0=V3[:, j_lo:jm, 1:33],
                        in1=B3[:, j_lo:jm, :])
                if j_hi == 33:          # last block: bias only here
                    eng.tensor_copy(
                        out=y_sb[:, 1024:1056], in_=brep_sb[:, 1024:1056])
                lo = max(1, j_lo)
                if j_hi > lo:           # += v[:, j-1, r+33]
                    eng.tensor_add(
                        out=Y3[:, lo:j_hi, :], in0=Y3[:, lo:j_hi, :],
                        in1=V3[:, lo - 1:j_hi - 1, 33:65])
                lo = max(2, j_lo)
                if j_hi > lo:           # r=0: += v[:, j-2, 65]
                    eng.tensor_add(
                        out=Y3[:, lo:j_hi, 0], in0=Y3[:, lo:j_hi, 0],
                        in1=V3[:, lo - 2:j_hi - 2, 65])
                hi = min(j_hi, 31)
                if hi > j_lo:           # r=31: += v[:, j+1, 0]
                    eng.tensor_add(
                        out=Y3[:, j_lo:hi, 31], in0=Y3[:, j_lo:hi, 31],
                        in1=V3[:, j_lo + 1:hi + 1, 0])

            # after vtrans(i) V[t] is final for t <= 4i+3, so y blocks
            # j < min(4i+3, 33) can assemble (block j reads up to t=j+1)
            y_wm = [0]

            def emit_rounds(i):
                if i < TC - 1:
                    j_hi = min(4 * i + 3, 33)
                    if j_hi > y_wm[0]:
                        emit_y_assembly(y_wm[0], j_hi, nc.gpsimd)
                        y_wm[0] = j_hi
                else:
                    # final round: split across gpsimd + vector (independent
                    # j ranges) to shorten the tail, each half stored as soon
                    # as it is assembled
                    mid = (y_wm[0] + 33 + 1) // 2
                    emit_y_assembly(y_wm[0], mid, nc.gpsimd)
                    nc.gpsimd.dma_start(out=y[:, 864:32 * mid],
                                        in_=y_sb[:, 864:32 * mid])
                    emit_y_assembly(mid, 33, nc.vector)
                    y_wm[0] = 33
                # progressive stores once column ranges are final (on the
                # gpsimd queue so their waits never block Sync x-dispatches)
                if y_wm[0] >= 15 and not store_done[0]:
                    nc.gpsimd.dma_start(out=y[:, 0:480], in_=y_sb[:, 0:480])
                    store_done[0] = True
                if y_wm[0] >= 27 and not store_done[1]:
                    nc.gpsimd.dma_start(out=y[:, 480:864], in_=y_sb[:, 480:864])
                    store_done[1] = True

            store_done = [False, False]

            prev = None          # (tcix, v_tiles) of the chunk awaiting fused
            emit_warmup()
            nc.sync.dma_start(out=b1c_sb[:], in_=b1c[:])
            emit_x_load(0, split_first=True)
            nc.sync.dma_start(out=w1t_sb[:], in_=w1t[:])
            emit_x_load_rest(0)
            emit_x_load(1)
            nc.sync.dma_start(out=g_sb[:], in_=g[:])
            emit_x_load_second_half(1)
            nc.sync.dma_start(out=ident_sb[:], in_=ident[:])
            nc.sync.dma_start(out=bvec_sb[:], in_=bvec[:])
            # bias row is tiny: broadcast on-device instead of loading 128
            # replicated rows over the contended early DMA window
            nc.gpsimd.partition_broadcast(brep_sb[:], bvec_sb[:])
            for tcix in range(TC):
                ht_tiles[tcix] = {
                    (cp, hc): ht_pool.tile(
                        [128, 2 * CW], FH,
                        tag=f"ht{hc}_{cp}", name=f"ht_{tcix}_{hc}_{cp}")
                    for cp in range(C // 2) for hc in range(HC)}
                if tcix + 2 < TC:
                    emit_x_load(tcix + 2)
                # interleave: MLP1(tcix, cp) with fused(tcix-1, c) so PE
                # always has matmul work while relu copies drain PSUM
                for c in range(C):
                    emit_mlp1(tcix, c)
                    if prev is not None:
                        emit_fused(prev[0], prev[1], c)
                if prev is not None:
                    emit_vtrans(prev[0], prev[1])
                    emit_rounds(prev[0])
                del xin_tiles[tcix]
                v_tiles = [
                    (lo, hi, var, vps_pool.tile(
                        [66, CW], FD, tag="v_ps", name=f"v_ps_{tcix}_{lo}"))
                    for (lo, hi, var) in chunk_ranges(tcix)]
                prev = (tcix, v_tiles)
            for c in range(C):
                emit_fused(prev[0], prev[1], c)
            emit_vtrans(prev[0], prev[1])
            emit_rounds(prev[0])

            mid_col = 32 * ((27 + 33 + 1) // 2)
            nc.sync.dma_start(out=y[:, mid_col:SIG_LEN],
                              in_=y_sb[:, mid_col:SIG_LEN])

    nc.finalize()
    return nc


def make_in_maps(inputs):
    """Per-core input maps (shared by kernel(), sim checks, and bench)."""
    x = np.asarray(inputs["encoder_output"], dtype=np.float32)
    W1 = np.asarray(inputs["W1"], np.float32)
    b1 = np.asarray(inputs["b1"], np.float32)

    G, Bvec = _host_prep(
        inputs["W1"], inputs["b1"], inputs["W2"], inputs["b2"],
        inputs["Winv"], inputs["binv"], inputs["Wconv"], inputs["bconv"])

    # pack G -> [128, HC*C*3*66]: g_sb[p, _g_col(hc,c,var)+m] = G[var, c, hc*128+p, m]
    g_pack = np.zeros((128, HC * C * 3 * 66), np.float32)
    for hc in range(HC):
        for c in range(C):
            for var in range(3):
                col = _g_col(hc, c, var)
                g_pack[:, col:col + 66] = G[var, c, hc * 128:(hc + 1) * 128, :]

    w1t = np.ascontiguousarray(W1.T).astype(np.float16)     # [E, H]
    g_pack = g_pack.astype(np.float16)
    b1c = np.ascontiguousarray(b1.reshape(HC, 128).T)       # [128, HC]
    bvec = np.ascontiguousarray(Bvec.reshape(1, SIG_LEN))
    ident = np.eye(128, dtype=np.float16)

    # [B,C,T,E] -> per-core [TC, E, (c, tl, b)] fp16 (device reads xT directly)
    xs = x.reshape(N_CORES, BL, C, TC, TL, E).transpose(0, 3, 5, 2, 4, 1)
    xs = np.ascontiguousarray(xs.astype(np.float16)).reshape(
        N_CORES, TC, E, C * CW)
    return [
        {
            "x": xs[i],
            "w1t": w1t, "b1c": b1c, "g": g_pack,
            "bvec": bvec, "ident": ident,
        }
        for i in range(N_CORES)
    ]


def kernel(**inputs) -> np.ndarray:
    if "nc" not in _CACHE:
        _CACHE["nc"] = _build_bass()
    nc = _CACHE["nc"]

    in_maps = make_in_maps(inputs)
    try:
        res = run_bass_kernel_spmd(nc, in_maps, core_ids=list(range(N_CORES)))
    except ModuleNotFoundError:
        # BASS_TRACE was set but the axon NTFF profile hook module is absent
        # in this environment; rerun without tracing.
        import os
        os.environ["BASS_NEVER_TRACE"] = "1"
        res = run_bass_kernel_spmd(nc, in_maps, core_ids=list(range(N_CORES)))
    _CACHE["last_result"] = res
    y = np.concatenate([r["y"] for r in res.results], axis=0)   # [B, 1056]
    return y.reshape(B, 1, SIG_LEN).astype(np.float32)


if __name__ == "__main__":
    rng = np.random.default_rng(0)
    ins = {
        "encoder_output": rng.standard_normal((B, C, T, E), dtype=np.float32),
        "W1": rng.standard_normal((H, E), dtype=np.float32) / np.sqrt(E),
        "b1": rng.standard_normal((H,), dtype=np.float32) / np.sqrt(E),
        "W2": rng.standard_normal((E, H), dtype=np.float32) / np.sqrt(H),
        "b2": rng.standard_normal((E,), dtype=np.float32) / np.sqrt(H),
        "Winv": rng.standard_normal((SEG_LEN, E), dtype=np.float32) / np.sqrt(E),
        "binv": rng.standard_normal((SEG_LEN,), dtype=np.float32) / np.sqrt(E),
        "Wconv": rng.standard_normal((1, C, 3), dtype=np.float32) / np.sqrt(C * 3),
        "bconv": rng.standard_normal((1,), dtype=np.float32) / np.sqrt(C * 3),
    }
    out = kernel(**ins)
    print("kernel output", out.shape, out.dtype)
